# revision 29
# baseline (speedup 1.0000x reference)
"""DeepseekV3 MLA attention prefill (S=1024, H=128 heads, HID=7168) on 8 TRN2
NeuronCores.

Sharding: tensor-parallel over heads (16 heads/core) for attention and the
output projection (host sums 8 partial outT). The fused a-projection is
COLUMN-sharded (v3): each core computes all 1024 rows for its own 320-column
slab of [q_a | kv_a | k_pe] (cores 0-5 own q columns, 6-7 own kv + k_pe,
64 pad columns on 0-6), so only ~19MB (full x^T + its wa slab) moves per
core instead of the 33MB replicated-wa scheme. The rms denominators cross
cores via an 8KB AllReduce; activations are produced directly in the
transposed [col, s] layout (stationary = wa chunks), so no PE transposes are
needed before the s-split AllGather pair.

Softmax normalization never touches the PE: gpsimd.partition_all_reduce +
fast DVE reciprocal + multiply, deferred one group. Causal windows start at
the diagonal. AV matmuls run one kc-step behind the score matmuls. kv_b
projections for 3 groups run right after the AllGather to cover its tail;
later groups are produced 2 groups ahead inside the loop.
"""
import math
import numpy as np
import ml_dtypes

import concourse.bass as bass
import concourse.mybir as mybir
import concourse.bacc as bacc
import concourse.bass_isa as bass_isa
import concourse.tile as tile
import concourse.bass_utils as bass_utils
from contextlib import ExitStack

F32 = mybir.dt.float32
BF16 = mybir.dt.bfloat16
AF = mybir.ActivationFunctionType
OP = mybir.AluOpType
RED = bass_isa.ReduceOp

N_CORES = 8
S = 1024
HID = 7168
H = 128
HG = H // N_CORES          # 16 heads per core
D_NOPE = 128
D_ROPE = 64
D_Q = D_NOPE + D_ROPE      # 192
D_V = 128
CQ = 1536                  # q lora rank
CKV = 512                  # kv lora rank
CA = CQ + CKV + D_ROPE     # 2112 fused a-proj cols
W_SL = 320                 # per-core a-proj column slab (256 owned + pad)
CC_A = HID // 128          # 56 contraction chunks for a-proj
SCALE = 1.0 / math.sqrt(D_Q)
EPS = 1e-6
G_HEADS = 2                # heads per group
N_GROUPS = HG // G_HEADS   # 8 groups
LAST_EXEC_NS = None

_CACHE = {}


def _dma_rows_to_3d(nc, dst, src_ap, n_chunks, p=128):
    """dst [p, n_chunks, w] <- src rows laid out as (chunk, p)."""
    try:
        nc.sync.dma_start(dst, src_ap.rearrange("(c p) s -> p c s", p=p))
    except Exception:
        for c in range(n_chunks):
            nc.sync.dma_start(dst[:, c, :], src_ap[c * p:(c + 1) * p, :])


def _build_nc():
    nc = bacc.Bacc("TRN2", target_bir_lowering=False, debug=False,
                   num_devices=N_CORES)

    xT = nc.dram_tensor("xT", [HID, S], BF16, kind="ExternalInput")
    wsl = nc.dram_tensor("wsl", [HID, W_SL], BF16, kind="ExternalInput")
    qbn = nc.dram_tensor("qbn", [CQ, HG * D_NOPE], BF16, kind="ExternalInput")
    qbp = nc.dram_tensor("qbp", [CQ, HG * D_ROPE], BF16, kind="ExternalInput")
    kvbk = nc.dram_tensor("kvbk", [CKV, HG * D_NOPE], BF16, kind="ExternalInput")
    kvbv = nc.dram_tensor("kvbv", [CKV, HG * D_V], BF16, kind="ExternalInput")
    ow = nc.dram_tensor("ow", [HG * D_V, HID], BF16, kind="ExternalInput")
    cos2t = nc.dram_tensor("cos2t", [128, S], F32, kind="ExternalInput")
    sin2tg = nc.dram_tensor("sin2tg", [128, S], F32, kind="ExternalInput")
    cosC = nc.dram_tensor("cosC", [D_ROPE, S], F32, kind="ExternalInput")
    sinC = nc.dram_tensor("sinC", [D_ROPE, S], F32, kind="ExternalInput")
    tri = nc.dram_tensor("tri", [128, 128], BF16, kind="ExternalInput")
    coremask = nc.dram_tensor("coremask", [N_CORES, 2], F32,
                               kind="ExternalInput")
    outT = nc.dram_tensor("outT", [HID, S], BF16, kind="ExternalOutput")

    with tile.TileContext(nc) as tc, ExitStack() as top:
        const = top.enter_context(tc.tile_pool(name="const", bufs=1))
        dram = top.enter_context(tc.tile_pool(name="dram", bufs=1, space="DRAM"))
        outsp = top.enter_context(tc.tile_pool(name="outsp", bufs=1))
        sbwq = top.enter_context(tc.tile_pool(name="sbwq", bufs=2))
        sbow = top.enter_context(tc.tile_pool(name="sbow", bufs=2))
        sbkw = top.enter_context(tc.tile_pool(name="sbkw", bufs=3))
        sbkv = top.enter_context(tc.tile_pool(name="sbkv", bufs=3))
        sbg = top.enter_context(tc.tile_pool(name="sbg", bufs=1))
        sbf = top.enter_context(tc.tile_pool(name="sbf", bufs=1))

        # ---- constants in SBUF ----
        tri_sb = const.tile([128, 128], BF16, tag="tri")
        nc.sync.dma_start(tri_sb[:], tri.ap())
        cos2t_sb = const.tile([128, S], F32, tag="cos2t")
        sin2tg_sb = const.tile([128, S], F32, tag="sin2tg")
        nc.sync.dma_start(cos2t_sb[:], cos2t.ap())
        nc.sync.dma_start(sin2tg_sb[:], sin2tg.ap())
        cosC_sb = const.tile([D_ROPE, S], F32, tag="cosC")
        sinC_sb = const.tile([D_ROPE, S], F32, tag="sinC")
        nc.sync.dma_start(cosC_sb[:], cosC.ap())
        nc.sync.dma_start(sinC_sb[:], sinC.ap())
        coremask_sb = const.tile([N_CORES, 2], F32, tag="coremask")
        nc.sync.dma_start(coremask_sb[:], coremask.ap())

        agiA = dram.tile([W_SL + 1, 512], BF16, tag="agiA")
        agiB = dram.tile([W_SL + 1, 512], BF16, tag="agiB")
        agoA = dram.tile([(W_SL + 1) * N_CORES, 512], BF16, tag="agoA",
                         addr_space="Shared")
        agoB = dram.tile([(W_SL + 1) * N_CORES, 512], BF16, tag="agoB",
                         addr_space="Shared")
        fkd = dram.tile([2, 512], F32, tag="fkd")

        # all 16 heads' attention outputs live in SBUF [dv=128, head, s]
        outs_sb = outsp.tile([128, HG, S], BF16, tag="outs")

        def load_group_qw(g):
            h0 = g * G_HEADS
            qbnw = sbwq.tile([128, CQ // 128, G_HEADS * 128], BF16,
                             tag="qbnw", name="qbnw")
            qbpw = sbwq.tile([128, CQ // 128, G_HEADS * 64], BF16,
                             tag="qbpw", name="qbpw")
            _dma_rows_to_3d(nc, qbnw[:],
                            qbn.ap()[:, h0 * 128:(h0 + G_HEADS) * 128], CQ // 128)
            _dma_rows_to_3d(nc, qbpw[:],
                            qbp.ap()[:, h0 * 64:(h0 + G_HEADS) * 64], CQ // 128)
            return qbnw, qbpw

        def load_group_kvw(g):
            h0 = g * G_HEADS
            kvbkw = sbkw.tile([128, CKV // 128, G_HEADS * 128], BF16,
                              tag="kvbkw", name="kvbkw")
            kvbvw = sbkw.tile([128, CKV // 128, G_HEADS * 128], BF16,
                              tag="kvbvw", name="kvbvw")
            _dma_rows_to_3d(nc, kvbkw[:],
                            kvbk.ap()[:, h0 * 128:(h0 + G_HEADS) * 128], CKV // 128)
            _dma_rows_to_3d(nc, kvbvw[:],
                            kvbv.ap()[:, h0 * 128:(h0 + G_HEADS) * 128], CKV // 128)
            return kvbkw, kvbvw

        def load_ow(nt):
            owt = sbow.tile([128, HG, 256], BF16, tag="owt", name="owt")
            _dma_rows_to_3d(nc, owt[:],
                            ow.ap()[:, nt * 256:(nt + 1) * 256], HG)
            return owt

        # kv_b output tiles: 3-slot sliding window (group g+2 is produced at
        # the end of group g's attention)
        kv_tiles = {}

        def emit_kvb_st(g, st, kvbkw, kvbvw, ckv):
            if g not in kv_tiles:
                kT_g = sbkv.tile([128, G_HEADS, S], BF16, tag="kT",
                                 name=f"kT{g}")
                v_g = sbkv.tile([128, 8, G_HEADS * 128], BF16, tag="v",
                                name=f"v{g}")
                kv_tiles[g] = (kT_g, v_g)
            kT_g, v_g = kv_tiles[g]
            for i in range(G_HEADS):
                p = ps_main.tile([128, 512], F32, tag="s", name="pkv")
                for c in range(CKV // 128):
                    nc.tensor.matmul(p[:], kvbkw[:, c, i * 128:(i + 1) * 128],
                                     ckv[st][:, c, :],
                                     start=(c == 0), stop=(c == CKV // 128 - 1))
                nc.vector.tensor_mul(kT_g[:, i, st * 512:(st + 1) * 512],
                                     p[:], fk_bc[:, st * 512:(st + 1) * 512])
            for sc in range(st * 4, st * 4 + 4):
                p = ps_main.tile([128, 512], F32, tag="s", name="pkv")
                nn = G_HEADS * 128
                for c in range(CKV // 128):
                    nc.tensor.matmul(
                        p[:, :nn],
                        ckv[st][:, c, (sc % 4) * 128:(sc % 4 + 1) * 128],
                        kvbvw[:, c, :],
                        start=(c == 0), stop=(c == CKV // 128 - 1))
                nc.vector.tensor_scalar_mul(v_g[:, sc, :], p[:, :nn],
                                            fkT[:, sc:sc + 1])

        # ================= Phase A: column-sharded a-proj ==================
        with ExitStack() as pa:
            sba = pa.enter_context(tc.tile_pool(name="sba", bufs=1))
            sbst = pa.enter_context(tc.tile_pool(name="sbst", bufs=4))
            psa = pa.enter_context(tc.tile_pool(name="psa", bufs=1, space="PSUM"))

            # psum accumulators: [col, s] layout, held across the whole pass
            pAc = [psa.tile([128, 512], F32, tag=f"pa{j}", name=f"pa{j}")
                   for j in range(4)]            # A-s0, A-s1, B-s0, B-s1
            pCc = [psa.tile([64, 512], F32, tag=f"pc{j}", name=f"pc{j}")
                   for j in range(2)]            # C-s0, C-s1

            for cc in range(CC_A):
                xt_t = sbst.tile([128, S], BF16, tag="xt", name="xt_t")
                nc.sync.dma_start(xt_t[:], xT.ap()[cc * 128:(cc + 1) * 128, :])
                ws_t = sbst.tile([128, W_SL], BF16, tag="ws", name="ws_t")
                nc.sync.dma_start(ws_t[:], wsl.ap()[cc * 128:(cc + 1) * 128, :])
                st_, sp_ = (cc == 0), (cc == CC_A - 1)
                for half in range(2):
                    sl = slice(half * 512, (half + 1) * 512)
                    nc.tensor.matmul(pAc[half][:], ws_t[:, 0:128], xt_t[:, sl],
                                     start=st_, stop=sp_)
                    nc.tensor.matmul(pAc[2 + half][:], ws_t[:, 128:256],
                                     xt_t[:, sl], start=st_, stop=sp_)
                    nc.tensor.matmul(pCc[half][:], ws_t[:, 256:320],
                                     xt_t[:, sl], start=st_, stop=sp_)

            # ---- rms denominators: each core's A/B columns are homogeneous
            # (all-q or all-kv), so a plain ones-column reduce gives this
            # core's own sums; routing to q vs kv happens receiver-side ----
            sqA = sba.tile([128, S], BF16, tag="sqA")
            sqB = sba.tile([128, S], BF16, tag="sqB")
            for half in range(2):
                sl = slice(half * 512, (half + 1) * 512)
                nc.scalar.activation(sqA[:, sl], pAc[half][:], AF.Square)
                nc.scalar.activation(sqB[:, sl], pAc[2 + half][:], AF.Square)
            psS = [psa.tile([1, 512], F32, tag=f"ss{j}", name=f"ss{j}")
                   for j in range(2)]
            for half in range(2):
                sl = slice(half * 512, (half + 1) * 512)
                nc.tensor.matmul(psS[half][:], tri_sb[:, 127:128], sqA[:, sl],
                                 start=True, stop=False)
                nc.tensor.matmul(psS[half][:], tri_sb[:, 127:128], sqB[:, sl],
                                 start=False, stop=True)
            # scale the raw sums down so their bf16 ride-along row keeps
            # precision (values ~1e3; 8-bit mantissa -> ~0.4% err, fine for
            # an rms denominator)
            sums_sb = sba.tile([1, S], BF16, tag="sums_sb")
            for half in range(2):
                nc.scalar.copy(sums_sb[0:1, half * 512:(half + 1) * 512],
                               psS[half][:])

            # ---- bf16 casts of the (unnormalized) activations ----
            bTA = sba.tile([128, S], BF16, tag="bTA")
            bTB = sba.tile([128, S], BF16, tag="bTB")
            for half in range(2):
                sl = slice(half * 512, (half + 1) * 512)
                nc.vector.tensor_copy(bTA[:, sl], pAc[half][:])
                nc.vector.tensor_copy(bTB[:, sl], pAc[2 + half][:])

            # ---- k_pe rope in transposed [d, s] layout (chunk C) ----
            # identity tables on cores 0-6 make this a no-op copy there
            kraw = sba.tile([64, S], F32, tag="kraw")
            rrC = sba.tile([64, S], F32, tag="rrC")
            bTC = sba.tile([64, S], BF16, tag="bTC")
            for half in range(2):
                sl = slice(half * 512, (half + 1) * 512)
                nc.vector.tensor_copy(kraw[:, sl], pCc[half][:])
            nc.vector.tensor_copy(rrC[0:32, :], kraw[32:64, :])
            nc.vector.tensor_copy(rrC[32:64, :], kraw[0:32, :])
            nc.vector.tensor_mul(rrC[:], rrC[:], sinC_sb[:])
            nc.vector.tensor_mul(kraw[:], kraw[:], cosC_sb[:])
            nc.vector.tensor_add(bTC[:], kraw[:], rrC[:])

            # ---- AllGather inputs (already transposed, raw) ----
            for agi, half in ((agiA, 0), (agiB, 1)):
                sl = slice(half * 512, (half + 1) * 512)
                nc.sync.dma_start(agi[0:128, :], bTA[:, sl])
                nc.sync.dma_start(agi[128:256, :], bTB[:, sl])
                nc.sync.dma_start(agi[256:320, :], bTC[:, sl])
                nc.sync.dma_start(agi[320:321, :], sums_sb[0:1, sl])
            nc.gpsimd.collective_compute(
                "AllGather", OP.bypass,
                replica_groups=[list(range(N_CORES))],
                ins=[agiA.opt()], outs=[agoA.opt()],
            )
            nc.gpsimd.collective_compute(
                "AllGather", OP.bypass,
                replica_groups=[list(range(N_CORES))],
                ins=[agiB.opt()], outs=[agoB.opt()],
            )

            # prefetches (FIFO DMA queues reach these behind the agi inputs,
            # well before the AllGathers complete)
            kvw_q = [load_group_kvw(0), load_group_kvw(1)]
            wq_pre = [load_group_qw(0), load_group_qw(1)]
            ow_pre = load_ow(0)

        ps_main = top.enter_context(tc.tile_pool(name="ps_main", bufs=3,
                                                 space="PSUM"))

        # ---- rms factors from the ride-along sums rows, one s-half at a
        # time as each AllGather lands: mask by core type, partition-reduce,
        # rsqrt, broadcast. fq_bc is folded into every qTn/qp psum copy; the
        # kv factor into the kT/v copies (fkT holds it transposed for v).
        fq_bc = sbf.tile([128, S], F32, tag="fq_bc")
        fk_bc = sbf.tile([128, S], F32, tag="fk_bc")
        fkT = sbf.tile([128, 8], F32, tag="fkT")
        with ExitStack() as pf:
            sbr = pf.enter_context(tc.tile_pool(name="sbr", bufs=2))
            for st, ago in ((0, agoA), (1, agoB)):
                sl = slice(st * 512, (st + 1) * 512)
                sums8 = sbr.tile([N_CORES, 512], BF16, tag="sums8",
                                 name="sums8")
                for c in range(N_CORES):
                    nc.sync.dma_start(sums8[c:c + 1, :],
                                      ago[c * (W_SL + 1) + W_SL:
                                          c * (W_SL + 1) + W_SL + 1, :])
                tq8 = sbr.tile([N_CORES, 512], F32, tag="tq8", name="tq8")
                tk8 = sbr.tile([N_CORES, 512], F32, tag="tk8", name="tk8")
                nc.vector.tensor_scalar_mul(tq8[:], sums8[:],
                                            coremask_sb[:, 0:1])
                nc.vector.tensor_scalar_mul(tk8[:], sums8[:],
                                            coremask_sb[:, 1:2])
                dq8 = sbr.tile([N_CORES, 512], F32, tag="dq8", name="dq8")
                dk8 = sbr.tile([N_CORES, 512], F32, tag="dk8", name="dk8")
                nc.gpsimd.partition_all_reduce(dq8[:], tq8[:], N_CORES,
                                               RED.add)
                nc.gpsimd.partition_all_reduce(dk8[:], tk8[:], N_CORES,
                                               RED.add)
                for d8, n in ((dq8, CQ), (dk8, CKV)):
                    nc.vector.tensor_scalar(d8[0:1, :], d8[0:1, :], 1.0 / n,
                                            EPS, OP.mult, OP.add)
                    nc.vector.reciprocal_approx_fast(d8[0:1, :], d8[0:1, :])
                    nc.scalar.activation(d8[0:1, :], d8[0:1, :], AF.Sqrt)
                nc.gpsimd.partition_broadcast(fq_bc[:, sl], dq8[0:1, :])
                nc.gpsimd.partition_broadcast(fk_bc[:, sl], dk8[0:1, :])
                # transposed kv factor [s-part, chunk] for the v copies,
                # via a DRAM bounce
                nc.sync.dma_start(fkd[st:st + 1, :], dk8[0:1, :])
                try:
                    nc.sync.dma_start(
                        fkT[:, st * 4:(st + 1) * 4],
                        fkd[st:st + 1, :].rearrange("o (c p) -> p (o c)",
                                                    p=128))
                except Exception:
                    for j in range(4):
                        nc.sync.dma_start(
                            fkT[:, st * 4 + j:st * 4 + j + 1],
                            fkd[st:st + 1, j * 128:(j + 1) * 128].rearrange(
                                "o p -> p o"))

        # ---- stitch the gathered activations ----
        # global row of col j on core c is c*320 + j; q cols sit on cores
        # 0-5 (2 aligned 128-chunks each), kv on 6-7, k_pe on core 7 rows
        # 256:320 (already rope'd, transposed)
        qct = []
        ckv = []
        kpe2 = sbg.tile([128, S], BF16, tag="kpe2")
        for st, ago in ((0, agoA), (1, agoB)):
            k_t = sbg.tile([128, CKV // 128, 512], BF16, tag=f"ckv{st}",
                           name=f"ckv{st}")
            for c in range(CKV // 128):
                base = (6 + c // 2) * (W_SL + 1) + (c % 2) * 128
                nc.sync.dma_start(k_t[:, c, :], ago[base:base + 128, :])
            ckv.append(k_t)
            base = 7 * (W_SL + 1) + 256
            nc.sync.dma_start(kpe2[0:64, st * 512:(st + 1) * 512],
                              ago[base:base + 64, :])
            nc.sync.dma_start(kpe2[64:128, st * 512:(st + 1) * 512],
                              ago[base:base + 64, :])
        for st, ago in ((0, agoA), (1, agoB)):
            q_t = sbg.tile([128, CQ // 128, 512], BF16, tag=f"qct{st}",
                           name=f"qct{st}")
            for c in range(CQ // 128):
                base = (c // 2) * (W_SL + 1) + (c % 2) * 128
                nc.sync.dma_start(q_t[:, c, :], ago[base:base + 128, :])
            qct.append(q_t)

        # kv_b for groups 0-2 covers the AllGather tail (st0 chains first so
        # they only wait on the first AllGather)
        kvw_q.append(load_group_kvw(2))
        for st in range(2):
            for g in range(3):
                emit_kvb_st(g, st, *kvw_q[g], ckv)

        # ================= Phase B: q_b projections + attention =============
        with ExitStack() as pb:
            sbh = pb.enter_context(tc.tile_pool(name="sbh", bufs=2))
            sbp = pb.enter_context(tc.tile_pool(name="sbp", bufs=1))
            sbpt = pb.enter_context(tc.tile_pool(name="sbpt", bufs=5))
            sbs = pb.enter_context(tc.tile_pool(name="sbs", bufs=2))
            sbn = pb.enter_context(tc.tile_pool(name="sbn", bufs=2))
            sbo = pb.enter_context(tc.tile_pool(name="sbo", bufs=3))
            ps_o = pb.enter_context(tc.tile_pool(name="ps_o", bufs=2, space="PSUM"))

            pending_norm = []

            def emit_norm_reduce():
                for idx, (h_idx, qt_, sums_, psum_o_) in enumerate(pending_norm):
                    den = sbn.tile([128, 512], F32, tag="den", name="den")
                    nc.gpsimd.partition_all_reduce(den[:], sums_[:], 128, RED.add)
                    pending_norm[idx] = (h_idx, qt_, den, psum_o_)

            def emit_norm_apply():
                while pending_norm:
                    h_idx, qt_, den, psum_o_ = pending_norm.pop(0)
                    rec = sbn.tile([128, 512], F32, tag="rec", name="rec")
                    nc.vector.reciprocal_approx_fast(rec[:], den[:])
                    nc.vector.tensor_mul(
                        outs_sb[:, h_idx, qt_ * 512:(qt_ + 1) * 512],
                        psum_o_[:], rec[:])

            for g in range(N_GROUPS):
                h0 = g * G_HEADS
                qbnw, qbpw = wq_pre[g] if g < 2 else load_group_qw(g)

                # --- q rope projection first so the DVE rope work is done
                # before the first rope-score matmul needs qTp ---
                qp_raw = sbp.tile([128, S], F32, tag="qp_raw", name="qp_raw")
                p0 = ps_main.tile([128, 512], F32, tag="s", name="p0")
                p1 = ps_main.tile([128, 512], F32, tag="s", name="p1")
                for c in range(CQ // 128):
                    nc.tensor.matmul(p0[:], qbpw[:, c, :], qct[0][:, c, :],
                                     start=(c == 0), stop=(c == CQ // 128 - 1))
                    nc.tensor.matmul(p1[:], qbpw[:, c, :], qct[1][:, c, :],
                                     start=(c == 0), stop=(c == CQ // 128 - 1))
                nc.vector.tensor_mul(qp_raw[:, 0:512], p0[:], fq_bc[:, 0:512])
                nc.vector.tensor_mul(qp_raw[:, 512:1024], p1[:],
                                     fq_bc[:, 512:1024])
                emit_norm_reduce()   # prev group's partition reduces (GpSimd)
                # rope on the head-pair tile: rows [0:64]=head h0, [64:128]=h0+1
                qTp = sbh.tile([128, S], BF16, tag="qTp")
                rs = sbp.tile([128, S], F32, tag="ropes")
                for hh in range(2):
                    sl = slice(hh * 512, (hh + 1) * 512)
                    for b in range(4):
                        r0 = b * 32
                        r1 = r0 + 32 if b % 2 == 0 else r0 - 32
                        nc.vector.tensor_copy(rs[r0:r0 + 32, sl], qp_raw[r1:r1 + 32, sl])
                    nc.vector.tensor_mul(rs[:, sl], rs[:, sl], sin2tg_sb[:, sl])
                    nc.vector.tensor_mul(qp_raw[:, sl], qp_raw[:, sl], cos2t_sb[:, sl])
                    nc.vector.tensor_add(qTp[:, sl], qp_raw[:, sl], rs[:, sl])

                # --- q nope projections; st-paired so each stationary is
                # loaded once per two 512-streams ---
                qTn = []
                for i in range(G_HEADS):
                    qt_t = sbh.tile([128, S], BF16, tag=f"qTn{i}", name=f"qTn{i}")
                    p0 = ps_main.tile([128, 512], F32, tag="s", name="p0")
                    p1 = ps_main.tile([128, 512], F32, tag="s", name="p1")
                    if g == 0 and i == 0:
                        # first chains after the AllGather: unpaired, so the
                        # st=0 chain starts as soon as half the stitch lands
                        for c in range(CQ // 128):
                            nc.tensor.matmul(p0[:], qbnw[:, c, 0:128],
                                             qct[0][:, c, :],
                                             start=(c == 0), stop=(c == CQ // 128 - 1))
                        for c in range(CQ // 128):
                            nc.tensor.matmul(p1[:], qbnw[:, c, 0:128],
                                             qct[1][:, c, :],
                                             start=(c == 0), stop=(c == CQ // 128 - 1))
                    else:
                        for c in range(CQ // 128):
                            nc.tensor.matmul(p0[:], qbnw[:, c, i * 128:(i + 1) * 128],
                                             qct[0][:, c, :],
                                             start=(c == 0), stop=(c == CQ // 128 - 1))
                            nc.tensor.matmul(p1[:], qbnw[:, c, i * 128:(i + 1) * 128],
                                             qct[1][:, c, :],
                                             start=(c == 0), stop=(c == CQ // 128 - 1))
                    nc.vector.tensor_mul(qt_t[:, 0:512], p0[:], fq_bc[:, 0:512])
                    nc.vector.tensor_mul(qt_t[:, 512:1024], p1[:],
                                         fq_bc[:, 512:1024])
                    qTn.append(qt_t)
                    if i == 0:
                        emit_norm_apply()  # prev group's recip+mul (DVE)

                kT_g, v_g = kv_tiles[g]

                # --- attention: heads interleaved, AV skewed one kc behind,
                # causal windows start at the diagonal ---
                for qt in range(2):
                    kmax = 4 * (qt + 1)
                    sums = [sbs.tile([128, 512], F32, tag=f"sums{i}", name=f"sums{i}")
                            for i in range(G_HEADS)]
                    psum_o = [ps_o.tile([128, 512], F32, tag=f"o{i}", name=f"po{i}")
                              for i in range(G_HEADS)]
                    pt = {}

                    def av_step(kc, last):
                        offp = max(0, (kc - 4 * qt)) * 128
                        for i in range(G_HEADS):
                            nc.tensor.matmul(psum_o[i][:, offp:512],
                                             v_g[:, kc, i * 128:(i + 1) * 128],
                                             pt[(i, kc)][:, offp:512],
                                             start=(kc == 0), stop=last,
                                             skip_group_check=True)

                    for kc in range(kmax):
                        off = max(0, (kc - 4 * qt)) * 128
                        qsl = slice(qt * 512 + off, (qt + 1) * 512)
                        for i in range(G_HEADS):
                            ps = ps_main.tile([128, 512], F32, tag="s", name="ps")
                            nc.tensor.matmul(ps[:, off:512],
                                             kT_g[:, i, kc * 128:(kc + 1) * 128],
                                             qTn[i][:, qsl],
                                             start=True, stop=False)
                            nc.tensor.matmul(ps[:, off:512],
                                             kpe2[i * 64:(i + 1) * 64,
                                                  kc * 128:(kc + 1) * 128],
                                             qTp[i * 64:(i + 1) * 64, qsl],
                                             start=False, stop=True)
                            p = sbpt.tile([128, 512], BF16, tag="pt", name="p")
                            nc.scalar.activation(p[:, off:512], ps[:, off:512],
                                                 AF.Exp, scale=SCALE)
                            if kc >= 4 * qt:
                                nc.vector.tensor_mul(p[:, off:off + 128],
                                                     p[:, off:off + 128], tri_sb[:])
                            if kc == 0:
                                nc.vector.tensor_copy(sums[i][:], p[:])
                            else:
                                nc.vector.tensor_add(sums[i][:, off:512],
                                                     sums[i][:, off:512],
                                                     p[:, off:512])
                            pt[(i, kc)] = p
                        if kc > 0:
                            av_step(kc - 1, last=False)
                    av_step(kmax - 1, last=True)
                    for i in range(G_HEADS):
                        pending_norm.append((h0 + i, qt, sums[i], psum_o[i]))

                # produce kv for group g+2 (slides the 3-buffer window)
                if g + 2 < N_GROUPS and g + 2 >= 3:
                    kvw = load_group_kvw(g + 2)
                    for st in range(2):
                        emit_kvb_st(g + 2, st, *kvw, ckv)

            emit_norm_reduce()
            emit_norm_apply()

            # ========= Phase C: partial output projection, out^T layout =====
            for nt in range(HID // 256):
                owt = ow_pre if nt == 0 else load_ow(nt)
                for ntl in range(2):
                    pA = ps_main.tile([128, 512], F32, tag="s", name="pA")
                    pB = ps_main.tile([128, 512], F32, tag="s", name="pB")
                    for hc in range(HG):
                        lhs = owt[:, hc, ntl * 128:(ntl + 1) * 128]
                        nc.tensor.matmul(pA[:], lhs, outs_sb[:, hc, 0:512],
                                         start=(hc == 0), stop=(hc == HG - 1))
                        nc.tensor.matmul(pB[:], lhs, outs_sb[:, hc, 512:1024],
                                         start=(hc == 0), stop=(hc == HG - 1))
                    for half, pp in ((0, pA), (1, pB)):
                        osb = sbo.tile([128, 512], BF16, tag="osb", name="osb")
                        nc.scalar.copy(osb[:], pp[:])
                        nc.sync.dma_start(
                            outT.ap()[nt * 256 + ntl * 128:nt * 256 + (ntl + 1) * 128,
                                      half * 512:(half + 1) * 512], osb[:])

    nc.compile()
    return nc


def _host_inputs(hidden_states, position_ids, q_a_weight, q_a_layernorm_weight,
                 q_b_weight, kv_a_weight, kv_a_layernorm_weight, kv_b_weight,
                 o_weight):
    bf16 = ml_dtypes.bfloat16
    x = np.asarray(hidden_states, np.float32).reshape(S, HID)
    pos = np.asarray(position_ids, np.float64).reshape(S)
    q_a_w = np.asarray(q_a_weight, np.float32)
    q_ln = np.asarray(q_a_layernorm_weight, np.float32)
    q_b_w = np.asarray(q_b_weight, np.float32)
    kv_a_w = np.asarray(kv_a_weight, np.float32)
    kv_ln = np.asarray(kv_a_layernorm_weight, np.float32)
    kv_b_w = np.asarray(kv_b_weight, np.float32)
    o_w = np.asarray(o_weight, np.float32)

    wa = np.concatenate([q_a_w, kv_a_w], axis=1)               # [HID, 2112]
    xT = np.ascontiguousarray(x.T).astype(bf16)                # [HID, S]

    # per-core 320-wide wa column slabs (cores 0-6: 256 owned + 64 pad)
    slabs = np.zeros((N_CORES, HID, W_SL), np.float32)
    for c in range(7):
        slabs[c, :, 0:256] = wa[:, c * 256:(c + 1) * 256]
    slabs[7] = wa[:, 1792:2112]

    # fold the rms-norm weights into the b-projections
    qb = (q_ln[:, None] * q_b_w).reshape(CQ, H, D_Q)
    kvb = (kv_ln[:, None] * kv_b_w).reshape(CKV, H, D_NOPE + D_V)

    # rope tables
    inv_freq = 1.0 / (10000.0 ** (np.arange(0, D_ROPE, 2, dtype=np.float64) / D_ROPE))
    freqs = pos[:, None] * inv_freq[None, :]                # [S, 32]
    emb = np.concatenate([freqs, freqs], axis=-1)           # [S, 64]
    cos = np.cos(emb).astype(np.float32)
    sin = np.sin(emb).astype(np.float32)
    sin_sg = np.concatenate([-sin[:, :32], sin[:, 32:]], axis=1)  # [S, 64]
    cosT = np.ascontiguousarray(cos.T)                      # [64, S]
    sinT_sg = np.ascontiguousarray(sin_sg.T)                # [64, S]
    cos2t = np.concatenate([cosT, cosT], axis=0)            # [128, S]
    sin2tg = np.concatenate([sinT_sg, sinT_sg], axis=0)     # [128, S]
    cos_id = np.ones((D_ROPE, S), np.float32)
    sin_id = np.zeros((D_ROPE, S), np.float32)

    # causal triangle for the diagonal 128x128 blocks: key row r valid for
    # query col c iff r <= c
    i = np.arange(128)[:, None]
    j = np.arange(128)[None, :]
    tri = (i <= j).astype(np.float32).astype(bf16)

    # which cores own q columns (0-5) vs kv columns (6-7)
    coremask = np.zeros((N_CORES, 2), np.float32)
    coremask[:6, 0] = 1.0
    coremask[6:, 1] = 1.0

    in_maps = []
    for c in range(N_CORES):
        hs = slice(c * HG, (c + 1) * HG)
        in_maps.append({
            "xT": xT,
            "wsl": slabs[c].astype(bf16),
            "qbn": np.ascontiguousarray(
                qb[:, hs, :D_NOPE].reshape(CQ, HG * D_NOPE)).astype(bf16),
            "qbp": np.ascontiguousarray(
                qb[:, hs, D_NOPE:].reshape(CQ, HG * D_ROPE)).astype(bf16),
            "kvbk": np.ascontiguousarray(
                kvb[:, hs, :D_NOPE].reshape(CKV, HG * D_NOPE)).astype(bf16),
            "kvbv": np.ascontiguousarray(
                kvb[:, hs, D_NOPE:].reshape(CKV, HG * D_V)).astype(bf16),
            "ow": np.ascontiguousarray(
                o_w[c * HG * D_V:(c + 1) * HG * D_V, :]).astype(bf16),
            "cos2t": cos2t,
            "sin2tg": sin2tg,
            "cosC": cosT if c == 7 else cos_id,
            "sinC": sinT_sg if c == 7 else sin_id,
            "tri": tri,
            "coremask": coremask,
        })
    return in_maps


def kernel(**inputs):
    global LAST_EXEC_NS
    trace = bool(inputs.pop("_trace", False))
    in_maps = _host_inputs(**inputs)
    if "nc" not in _CACHE:
        _CACHE["nc"] = _build_nc()
    nc = _CACHE["nc"]
    res = bass_utils.run_bass_kernel_spmd(
        nc, in_maps, core_ids=list(range(N_CORES)), trace=trace)
    LAST_EXEC_NS = res.exec_time_ns
    total = np.zeros((HID, S), np.float64)
    for c in range(N_CORES):
        total += res.results[c]["outT"].astype(np.float64)
    return np.ascontiguousarray(total.T).astype(np.float32).reshape(1, 1, S, HID)


# revision 30
# speedup vs baseline: 1.0046x; 1.0046x over previous
"""DeepseekV3 MLA attention prefill (S=1024, H=128 heads, HID=7168) on 8 TRN2
NeuronCores.

Sharding: tensor-parallel over heads (16 heads/core) for attention and the
output projection (host sums 8 partial outT). The fused a-projection is
COLUMN-sharded (v3): each core computes all 1024 rows for its own 320-column
slab of [q_a | kv_a | k_pe] (cores 0-5 own q columns, 6-7 own kv + k_pe,
64 pad columns on 0-6), so only ~19MB (full x^T + its wa slab) moves per
core instead of the 33MB replicated-wa scheme. The rms denominators cross
cores via an 8KB AllReduce; activations are produced directly in the
transposed [col, s] layout (stationary = wa chunks), so no PE transposes are
needed before the s-split AllGather pair.

Softmax normalization never touches the PE: gpsimd.partition_all_reduce +
fast DVE reciprocal + multiply, deferred one group. Causal windows start at
the diagonal. AV matmuls run one kc-step behind the score matmuls. kv_b
projections for 3 groups run right after the AllGather to cover its tail;
later groups are produced 2 groups ahead inside the loop.
"""
import math
import numpy as np
import ml_dtypes

import concourse.bass as bass
import concourse.mybir as mybir
import concourse.bacc as bacc
import concourse.bass_isa as bass_isa
import concourse.tile as tile
import concourse.bass_utils as bass_utils
from contextlib import ExitStack

F32 = mybir.dt.float32
BF16 = mybir.dt.bfloat16
AF = mybir.ActivationFunctionType
OP = mybir.AluOpType
RED = bass_isa.ReduceOp

N_CORES = 8
S = 1024
HID = 7168
H = 128
HG = H // N_CORES          # 16 heads per core
D_NOPE = 128
D_ROPE = 64
D_Q = D_NOPE + D_ROPE      # 192
D_V = 128
CQ = 1536                  # q lora rank
CKV = 512                  # kv lora rank
CA = CQ + CKV + D_ROPE     # 2112 fused a-proj cols
W_SL = 320                 # per-core a-proj column slab (256 owned + pad)
CC_A = HID // 128          # 56 contraction chunks for a-proj
SCALE = 1.0 / math.sqrt(D_Q)
EPS = 1e-6
G_HEADS = 2                # heads per group
N_GROUPS = HG // G_HEADS   # 8 groups
LAST_EXEC_NS = None

_CACHE = {}


def _dma_rows_to_3d(nc, dst, src_ap, n_chunks, p=128):
    """dst [p, n_chunks, w] <- src rows laid out as (chunk, p)."""
    try:
        nc.sync.dma_start(dst, src_ap.rearrange("(c p) s -> p c s", p=p))
    except Exception:
        for c in range(n_chunks):
            nc.sync.dma_start(dst[:, c, :], src_ap[c * p:(c + 1) * p, :])


def _build_nc():
    nc = bacc.Bacc("TRN2", target_bir_lowering=False, debug=False,
                   num_devices=N_CORES)

    xT = nc.dram_tensor("xT", [HID, S], BF16, kind="ExternalInput")
    wsl = nc.dram_tensor("wsl", [HID, W_SL], BF16, kind="ExternalInput")
    qbn = nc.dram_tensor("qbn", [CQ, HG * D_NOPE], BF16, kind="ExternalInput")
    qbp = nc.dram_tensor("qbp", [CQ, HG * D_ROPE], BF16, kind="ExternalInput")
    kvbk = nc.dram_tensor("kvbk", [CKV, HG * D_NOPE], BF16, kind="ExternalInput")
    kvbv = nc.dram_tensor("kvbv", [CKV, HG * D_V], BF16, kind="ExternalInput")
    ow = nc.dram_tensor("ow", [HG * D_V, HID], BF16, kind="ExternalInput")
    cos2t = nc.dram_tensor("cos2t", [128, S], F32, kind="ExternalInput")
    sin2tg = nc.dram_tensor("sin2tg", [128, S], F32, kind="ExternalInput")
    cosC = nc.dram_tensor("cosC", [D_ROPE, S], F32, kind="ExternalInput")
    sinC = nc.dram_tensor("sinC", [D_ROPE, S], F32, kind="ExternalInput")
    tri = nc.dram_tensor("tri", [128, 128], BF16, kind="ExternalInput")
    coremask = nc.dram_tensor("coremask", [N_CORES, 2], F32,
                               kind="ExternalInput")
    outT = nc.dram_tensor("outT", [HID, S], BF16, kind="ExternalOutput")

    with tile.TileContext(nc) as tc, ExitStack() as top:
        const = top.enter_context(tc.tile_pool(name="const", bufs=1))
        dram = top.enter_context(tc.tile_pool(name="dram", bufs=1, space="DRAM"))
        outsp = top.enter_context(tc.tile_pool(name="outsp", bufs=1))
        sbwq = top.enter_context(tc.tile_pool(name="sbwq", bufs=2))
        sbow = top.enter_context(tc.tile_pool(name="sbow", bufs=2))
        sbkw = top.enter_context(tc.tile_pool(name="sbkw", bufs=3))
        sbkv = top.enter_context(tc.tile_pool(name="sbkv", bufs=3))
        sbg = top.enter_context(tc.tile_pool(name="sbg", bufs=1))
        sbf = top.enter_context(tc.tile_pool(name="sbf", bufs=1))

        # ---- constants in SBUF ----
        tri_sb = const.tile([128, 128], BF16, tag="tri")
        nc.sync.dma_start(tri_sb[:], tri.ap())
        cos2t_sb = const.tile([128, S], F32, tag="cos2t")
        sin2tg_sb = const.tile([128, S], F32, tag="sin2tg")
        nc.sync.dma_start(cos2t_sb[:], cos2t.ap())
        nc.sync.dma_start(sin2tg_sb[:], sin2tg.ap())
        cosC_sb = const.tile([D_ROPE, S], F32, tag="cosC")
        sinC_sb = const.tile([D_ROPE, S], F32, tag="sinC")
        nc.sync.dma_start(cosC_sb[:], cosC.ap())
        nc.sync.dma_start(sinC_sb[:], sinC.ap())
        coremask_sb = const.tile([N_CORES, 2], F32, tag="coremask")
        nc.sync.dma_start(coremask_sb[:], coremask.ap())

        agiA = dram.tile([W_SL + 1, 512], BF16, tag="agiA")
        agiB = dram.tile([W_SL + 1, 512], BF16, tag="agiB")
        agoA = dram.tile([(W_SL + 1) * N_CORES, 512], BF16, tag="agoA",
                         addr_space="Shared")
        agoB = dram.tile([(W_SL + 1) * N_CORES, 512], BF16, tag="agoB",
                         addr_space="Shared")

        # all 16 heads' attention outputs live in SBUF [dv=128, head, s]
        outs_sb = outsp.tile([128, HG, S], BF16, tag="outs")

        def load_group_qw(g):
            h0 = g * G_HEADS
            qbnw = sbwq.tile([128, CQ // 128, G_HEADS * 128], BF16,
                             tag="qbnw", name="qbnw")
            qbpw = sbwq.tile([128, CQ // 128, G_HEADS * 64], BF16,
                             tag="qbpw", name="qbpw")
            _dma_rows_to_3d(nc, qbnw[:],
                            qbn.ap()[:, h0 * 128:(h0 + G_HEADS) * 128], CQ // 128)
            _dma_rows_to_3d(nc, qbpw[:],
                            qbp.ap()[:, h0 * 64:(h0 + G_HEADS) * 64], CQ // 128)
            return qbnw, qbpw

        def load_group_kvw(g):
            h0 = g * G_HEADS
            kvbkw = sbkw.tile([128, CKV // 128, G_HEADS * 128], BF16,
                              tag="kvbkw", name="kvbkw")
            kvbvw = sbkw.tile([128, CKV // 128, G_HEADS * 128], BF16,
                              tag="kvbvw", name="kvbvw")
            _dma_rows_to_3d(nc, kvbkw[:],
                            kvbk.ap()[:, h0 * 128:(h0 + G_HEADS) * 128], CKV // 128)
            _dma_rows_to_3d(nc, kvbvw[:],
                            kvbv.ap()[:, h0 * 128:(h0 + G_HEADS) * 128], CKV // 128)
            return kvbkw, kvbvw

        def load_ow(nt):
            owt = sbow.tile([128, HG, 256], BF16, tag="owt", name="owt")
            _dma_rows_to_3d(nc, owt[:],
                            ow.ap()[:, nt * 256:(nt + 1) * 256], HG)
            return owt

        # kv_b output tiles: 3-slot sliding window (group g+2 is produced at
        # the end of group g's attention)
        kv_tiles = {}

        def emit_kvb_st(g, st, kvbkw, kvbvw, ckv):
            if g not in kv_tiles:
                kT_g = sbkv.tile([128, G_HEADS, S], BF16, tag="kT",
                                 name=f"kT{g}")
                v_g = sbkv.tile([128, 8, G_HEADS * 128], BF16, tag="v",
                                name=f"v{g}")
                kv_tiles[g] = (kT_g, v_g)
            kT_g, v_g = kv_tiles[g]
            for i in range(G_HEADS):
                p = ps_main.tile([128, 512], F32, tag="s", name="pkv")
                for c in range(CKV // 128):
                    nc.tensor.matmul(p[:], kvbkw[:, c, i * 128:(i + 1) * 128],
                                     ckv[st][:, c, :],
                                     start=(c == 0), stop=(c == CKV // 128 - 1))
                nc.scalar.copy(kT_g[:, i, st * 512:(st + 1) * 512], p[:])
            for sc in range(st * 4, st * 4 + 4):
                p = ps_main.tile([128, 512], F32, tag="s", name="pkv")
                nn = G_HEADS * 128
                for c in range(CKV // 128):
                    nc.tensor.matmul(
                        p[:, :nn],
                        ckv[st][:, c, (sc % 4) * 128:(sc % 4 + 1) * 128],
                        kvbvw[:, c, :],
                        start=(c == 0), stop=(c == CKV // 128 - 1))
                nc.scalar.copy(v_g[:, sc, :], p[:, :nn])

        # ================= Phase A: column-sharded a-proj ==================
        with ExitStack() as pa:
            sba = pa.enter_context(tc.tile_pool(name="sba", bufs=1))
            sbst = pa.enter_context(tc.tile_pool(name="sbst", bufs=4))
            psa = pa.enter_context(tc.tile_pool(name="psa", bufs=1, space="PSUM"))

            # psum accumulators: [col, s] layout, held across the whole pass
            pAc = [psa.tile([128, 512], F32, tag=f"pa{j}", name=f"pa{j}")
                   for j in range(4)]            # A-s0, A-s1, B-s0, B-s1
            pCc = [psa.tile([64, 512], F32, tag=f"pc{j}", name=f"pc{j}")
                   for j in range(2)]            # C-s0, C-s1

            for cc in range(CC_A):
                xt_t = sbst.tile([128, S], BF16, tag="xt", name="xt_t")
                nc.sync.dma_start(xt_t[:], xT.ap()[cc * 128:(cc + 1) * 128, :])
                ws_t = sbst.tile([128, W_SL], BF16, tag="ws", name="ws_t")
                nc.sync.dma_start(ws_t[:], wsl.ap()[cc * 128:(cc + 1) * 128, :])
                st_, sp_ = (cc == 0), (cc == CC_A - 1)
                for half in range(2):
                    sl = slice(half * 512, (half + 1) * 512)
                    nc.tensor.matmul(pAc[half][:], ws_t[:, 0:128], xt_t[:, sl],
                                     start=st_, stop=sp_)
                    nc.tensor.matmul(pAc[2 + half][:], ws_t[:, 128:256],
                                     xt_t[:, sl], start=st_, stop=sp_)
                    nc.tensor.matmul(pCc[half][:], ws_t[:, 256:320],
                                     xt_t[:, sl], start=st_, stop=sp_)

            # ---- rms denominators: each core's A/B columns are homogeneous
            # (all-q or all-kv), so a plain ones-column reduce gives this
            # core's own sums; routing to q vs kv happens receiver-side ----
            sqA = sba.tile([128, S], BF16, tag="sqA")
            sqB = sba.tile([128, S], BF16, tag="sqB")
            for half in range(2):
                sl = slice(half * 512, (half + 1) * 512)
                nc.scalar.activation(sqA[:, sl], pAc[half][:], AF.Square)
                nc.scalar.activation(sqB[:, sl], pAc[2 + half][:], AF.Square)
            psS = [psa.tile([1, 512], F32, tag=f"ss{j}", name=f"ss{j}")
                   for j in range(2)]
            for half in range(2):
                sl = slice(half * 512, (half + 1) * 512)
                nc.tensor.matmul(psS[half][:], tri_sb[:, 127:128], sqA[:, sl],
                                 start=True, stop=False)
                nc.tensor.matmul(psS[half][:], tri_sb[:, 127:128], sqB[:, sl],
                                 start=False, stop=True)
            # scale the raw sums down so their bf16 ride-along row keeps
            # precision (values ~1e3; 8-bit mantissa -> ~0.4% err, fine for
            # an rms denominator)
            sums_sb = sba.tile([1, S], BF16, tag="sums_sb")
            for half in range(2):
                nc.scalar.copy(sums_sb[0:1, half * 512:(half + 1) * 512],
                               psS[half][:])

            # ---- bf16 casts of the (unnormalized) activations ----
            bTA = sba.tile([128, S], BF16, tag="bTA")
            bTB = sba.tile([128, S], BF16, tag="bTB")
            for half in range(2):
                sl = slice(half * 512, (half + 1) * 512)
                nc.vector.tensor_copy(bTA[:, sl], pAc[half][:])
                nc.vector.tensor_copy(bTB[:, sl], pAc[2 + half][:])

            # ---- k_pe rope in transposed [d, s] layout (chunk C) ----
            # identity tables on cores 0-6 make this a no-op copy there
            kraw = sba.tile([64, S], F32, tag="kraw")
            rrC = sba.tile([64, S], F32, tag="rrC")
            bTC = sba.tile([64, S], BF16, tag="bTC")
            for half in range(2):
                sl = slice(half * 512, (half + 1) * 512)
                nc.vector.tensor_copy(kraw[:, sl], pCc[half][:])
            nc.vector.tensor_copy(rrC[0:32, :], kraw[32:64, :])
            nc.vector.tensor_copy(rrC[32:64, :], kraw[0:32, :])
            nc.vector.tensor_mul(rrC[:], rrC[:], sinC_sb[:])
            nc.vector.tensor_mul(kraw[:], kraw[:], cosC_sb[:])
            nc.vector.tensor_add(bTC[:], kraw[:], rrC[:])

            # ---- AllGather inputs (already transposed, raw) ----
            for agi, half in ((agiA, 0), (agiB, 1)):
                sl = slice(half * 512, (half + 1) * 512)
                nc.sync.dma_start(agi[0:128, :], bTA[:, sl])
                nc.sync.dma_start(agi[128:256, :], bTB[:, sl])
                nc.sync.dma_start(agi[256:320, :], bTC[:, sl])
                nc.sync.dma_start(agi[320:321, :], sums_sb[0:1, sl])
            nc.gpsimd.collective_compute(
                "AllGather", OP.bypass,
                replica_groups=[list(range(N_CORES))],
                ins=[agiA.opt()], outs=[agoA.opt()],
            )
            nc.gpsimd.collective_compute(
                "AllGather", OP.bypass,
                replica_groups=[list(range(N_CORES))],
                ins=[agiB.opt()], outs=[agoB.opt()],
            )

            # prefetches (FIFO DMA queues reach these behind the agi inputs,
            # well before the AllGathers complete)
            kvw_q = [load_group_kvw(0), load_group_kvw(1)]
            wq_pre = [load_group_qw(0), load_group_qw(1)]
            ow_pre = load_ow(0)

        ps_main = top.enter_context(tc.tile_pool(name="ps_main", bufs=3,
                                                 space="PSUM"))

        # ---- rms factors from the ride-along sums rows, one s-half at a
        # time as each AllGather lands: mask by core type, partition-reduce,
        # rsqrt, broadcast. fq_bc is folded into every qTn/qp psum copy; the
        # kv factor into the kT/v copies (fkT holds it transposed for v).
        fq_bc = sbf.tile([128, S], F32, tag="fq_bc")
        fk_bc = sbf.tile([128, S], F32, tag="fk_bc")
        with ExitStack() as pf:
            sbr = pf.enter_context(tc.tile_pool(name="sbr", bufs=2))
            for st, ago in ((0, agoA), (1, agoB)):
                sl = slice(st * 512, (st + 1) * 512)
                sums8 = sbr.tile([N_CORES, 512], BF16, tag="sums8",
                                 name="sums8")
                for c in range(N_CORES):
                    nc.sync.dma_start(sums8[c:c + 1, :],
                                      ago[c * (W_SL + 1) + W_SL:
                                          c * (W_SL + 1) + W_SL + 1, :])
                tq8 = sbr.tile([N_CORES, 512], F32, tag="tq8", name="tq8")
                tk8 = sbr.tile([N_CORES, 512], F32, tag="tk8", name="tk8")
                nc.vector.tensor_scalar_mul(tq8[:], sums8[:],
                                            coremask_sb[:, 0:1])
                nc.vector.tensor_scalar_mul(tk8[:], sums8[:],
                                            coremask_sb[:, 1:2])
                dq8 = sbr.tile([N_CORES, 512], F32, tag="dq8", name="dq8")
                dk8 = sbr.tile([N_CORES, 512], F32, tag="dk8", name="dk8")
                nc.gpsimd.partition_all_reduce(dq8[:], tq8[:], N_CORES,
                                               RED.add)
                nc.gpsimd.partition_all_reduce(dk8[:], tk8[:], N_CORES,
                                               RED.add)
                for d8, n in ((dq8, CQ), (dk8, CKV)):
                    nc.vector.tensor_scalar(d8[0:1, :], d8[0:1, :], 1.0 / n,
                                            EPS, OP.mult, OP.add)
                    nc.vector.reciprocal_approx_fast(d8[0:1, :], d8[0:1, :])
                    nc.scalar.activation(d8[0:1, :], d8[0:1, :], AF.Sqrt)
                nc.gpsimd.partition_broadcast(fq_bc[:, sl], dq8[0:1, :])
                nc.gpsimd.partition_broadcast(fk_bc[:, sl], dk8[0:1, :])

        # ---- stitch the gathered activations ----
        # global row of col j on core c is c*320 + j; q cols sit on cores
        # 0-5 (2 aligned 128-chunks each), kv on 6-7, k_pe on core 7 rows
        # 256:320 (already rope'd, transposed)
        qct = []
        ckv = []
        kpe2 = sbg.tile([128, S], BF16, tag="kpe2")
        for st, ago in ((0, agoA), (1, agoB)):
            k_t = sbg.tile([128, CKV // 128, 512], BF16, tag=f"ckv{st}",
                           name=f"ckv{st}")
            for c in range(CKV // 128):
                base = (6 + c // 2) * (W_SL + 1) + (c % 2) * 128
                nc.sync.dma_start(k_t[:, c, :], ago[base:base + 128, :])
            # fold the kv rms factor into the stitched tiles once
            for c in range(CKV // 128):
                nc.vector.tensor_mul(k_t[:, c, :], k_t[:, c, :],
                                     fk_bc[:, st * 512:(st + 1) * 512])
            ckv.append(k_t)
            base = 7 * (W_SL + 1) + 256
            nc.sync.dma_start(kpe2[0:64, st * 512:(st + 1) * 512],
                              ago[base:base + 64, :])
            nc.sync.dma_start(kpe2[64:128, st * 512:(st + 1) * 512],
                              ago[base:base + 64, :])
        for st, ago in ((0, agoA), (1, agoB)):
            q_t = sbg.tile([128, CQ // 128, 512], BF16, tag=f"qct{st}",
                           name=f"qct{st}")
            for c in range(CQ // 128):
                base = (c // 2) * (W_SL + 1) + (c % 2) * 128
                nc.sync.dma_start(q_t[:, c, :], ago[base:base + 128, :])
            qct.append(q_t)

        # kv_b for groups 0-2 covers the AllGather tail (st0 chains first so
        # they only wait on the first AllGather)
        kvw_q.append(load_group_kvw(2))
        for st in range(2):
            for g in range(3):
                emit_kvb_st(g, st, *kvw_q[g], ckv)

        # ================= Phase B: q_b projections + attention =============
        with ExitStack() as pb:
            sbh = pb.enter_context(tc.tile_pool(name="sbh", bufs=2))
            sbp = pb.enter_context(tc.tile_pool(name="sbp", bufs=1))
            sbpt = pb.enter_context(tc.tile_pool(name="sbpt", bufs=5))
            sbs = pb.enter_context(tc.tile_pool(name="sbs", bufs=2))
            sbn = pb.enter_context(tc.tile_pool(name="sbn", bufs=2))
            sbo = pb.enter_context(tc.tile_pool(name="sbo", bufs=3))
            ps_o = pb.enter_context(tc.tile_pool(name="ps_o", bufs=2, space="PSUM"))

            pending_norm = []

            def emit_norm_reduce():
                for idx, (h_idx, qt_, sums_, psum_o_) in enumerate(pending_norm):
                    den = sbn.tile([128, 512], F32, tag="den", name="den")
                    nc.gpsimd.partition_all_reduce(den[:], sums_[:], 128, RED.add)
                    pending_norm[idx] = (h_idx, qt_, den, psum_o_)

            def emit_norm_apply():
                while pending_norm:
                    h_idx, qt_, den, psum_o_ = pending_norm.pop(0)
                    rec = sbn.tile([128, 512], F32, tag="rec", name="rec")
                    nc.vector.reciprocal_approx_fast(rec[:], den[:])
                    nc.vector.tensor_mul(
                        outs_sb[:, h_idx, qt_ * 512:(qt_ + 1) * 512],
                        psum_o_[:], rec[:])

            for g in range(N_GROUPS):
                h0 = g * G_HEADS
                qbnw, qbpw = wq_pre[g] if g < 2 else load_group_qw(g)

                # --- q rope projection first so the DVE rope work is done
                # before the first rope-score matmul needs qTp ---
                qp_raw = sbp.tile([128, S], F32, tag="qp_raw", name="qp_raw")
                p0 = ps_main.tile([128, 512], F32, tag="s", name="p0")
                p1 = ps_main.tile([128, 512], F32, tag="s", name="p1")
                for c in range(CQ // 128):
                    nc.tensor.matmul(p0[:], qbpw[:, c, :], qct[0][:, c, :],
                                     start=(c == 0), stop=(c == CQ // 128 - 1))
                    nc.tensor.matmul(p1[:], qbpw[:, c, :], qct[1][:, c, :],
                                     start=(c == 0), stop=(c == CQ // 128 - 1))
                nc.vector.tensor_mul(qp_raw[:, 0:512], p0[:], fq_bc[:, 0:512])
                nc.vector.tensor_mul(qp_raw[:, 512:1024], p1[:],
                                     fq_bc[:, 512:1024])
                emit_norm_reduce()   # prev group's partition reduces (GpSimd)
                # rope on the head-pair tile: rows [0:64]=head h0, [64:128]=h0+1
                qTp = sbh.tile([128, S], BF16, tag="qTp")
                rs = sbp.tile([128, S], F32, tag="ropes")
                for hh in range(2):
                    sl = slice(hh * 512, (hh + 1) * 512)
                    for b in range(4):
                        r0 = b * 32
                        r1 = r0 + 32 if b % 2 == 0 else r0 - 32
                        nc.vector.tensor_copy(rs[r0:r0 + 32, sl], qp_raw[r1:r1 + 32, sl])
                    nc.vector.tensor_mul(rs[:, sl], rs[:, sl], sin2tg_sb[:, sl])
                    nc.vector.tensor_mul(qp_raw[:, sl], qp_raw[:, sl], cos2t_sb[:, sl])
                    nc.vector.tensor_add(qTp[:, sl], qp_raw[:, sl], rs[:, sl])

                # --- q nope projections; st-paired so each stationary is
                # loaded once per two 512-streams ---
                qTn = []
                for i in range(G_HEADS):
                    qt_t = sbh.tile([128, S], BF16, tag=f"qTn{i}", name=f"qTn{i}")
                    p0 = ps_main.tile([128, 512], F32, tag="s", name="p0")
                    p1 = ps_main.tile([128, 512], F32, tag="s", name="p1")
                    if g == 0 and i == 0:
                        # first chains after the AllGather: unpaired, so the
                        # st=0 chain starts as soon as half the stitch lands
                        for c in range(CQ // 128):
                            nc.tensor.matmul(p0[:], qbnw[:, c, 0:128],
                                             qct[0][:, c, :],
                                             start=(c == 0), stop=(c == CQ // 128 - 1))
                        for c in range(CQ // 128):
                            nc.tensor.matmul(p1[:], qbnw[:, c, 0:128],
                                             qct[1][:, c, :],
                                             start=(c == 0), stop=(c == CQ // 128 - 1))
                    else:
                        for c in range(CQ // 128):
                            nc.tensor.matmul(p0[:], qbnw[:, c, i * 128:(i + 1) * 128],
                                             qct[0][:, c, :],
                                             start=(c == 0), stop=(c == CQ // 128 - 1))
                            nc.tensor.matmul(p1[:], qbnw[:, c, i * 128:(i + 1) * 128],
                                             qct[1][:, c, :],
                                             start=(c == 0), stop=(c == CQ // 128 - 1))
                    nc.vector.tensor_mul(qt_t[:, 0:512], p0[:], fq_bc[:, 0:512])
                    nc.vector.tensor_mul(qt_t[:, 512:1024], p1[:],
                                         fq_bc[:, 512:1024])
                    qTn.append(qt_t)
                    if i == 0:
                        emit_norm_apply()  # prev group's recip+mul (DVE)

                kT_g, v_g = kv_tiles[g]

                # --- attention: heads interleaved, AV skewed one kc behind,
                # causal windows start at the diagonal ---
                for qt in range(2):
                    kmax = 4 * (qt + 1)
                    sums = [sbs.tile([128, 512], F32, tag=f"sums{i}", name=f"sums{i}")
                            for i in range(G_HEADS)]
                    psum_o = [ps_o.tile([128, 512], F32, tag=f"o{i}", name=f"po{i}")
                              for i in range(G_HEADS)]
                    pt = {}

                    def av_step(kc, last):
                        offp = max(0, (kc - 4 * qt)) * 128
                        for i in range(G_HEADS):
                            nc.tensor.matmul(psum_o[i][:, offp:512],
                                             v_g[:, kc, i * 128:(i + 1) * 128],
                                             pt[(i, kc)][:, offp:512],
                                             start=(kc == 0), stop=last,
                                             skip_group_check=True)

                    for kc in range(kmax):
                        off = max(0, (kc - 4 * qt)) * 128
                        qsl = slice(qt * 512 + off, (qt + 1) * 512)
                        for i in range(G_HEADS):
                            ps = ps_main.tile([128, 512], F32, tag="s", name="ps")
                            nc.tensor.matmul(ps[:, off:512],
                                             kT_g[:, i, kc * 128:(kc + 1) * 128],
                                             qTn[i][:, qsl],
                                             start=True, stop=False)
                            nc.tensor.matmul(ps[:, off:512],
                                             kpe2[i * 64:(i + 1) * 64,
                                                  kc * 128:(kc + 1) * 128],
                                             qTp[i * 64:(i + 1) * 64, qsl],
                                             start=False, stop=True)
                            p = sbpt.tile([128, 512], BF16, tag="pt", name="p")
                            nc.scalar.activation(p[:, off:512], ps[:, off:512],
                                                 AF.Exp, scale=SCALE)
                            if kc >= 4 * qt:
                                nc.vector.tensor_mul(p[:, off:off + 128],
                                                     p[:, off:off + 128], tri_sb[:])
                            if kc == 0:
                                nc.vector.tensor_copy(sums[i][:], p[:])
                            else:
                                nc.vector.tensor_add(sums[i][:, off:512],
                                                     sums[i][:, off:512],
                                                     p[:, off:512])
                            pt[(i, kc)] = p
                        if kc > 0:
                            av_step(kc - 1, last=False)
                    av_step(kmax - 1, last=True)
                    for i in range(G_HEADS):
                        pending_norm.append((h0 + i, qt, sums[i], psum_o[i]))

                # produce kv for group g+2 (slides the 3-buffer window)
                if g + 2 < N_GROUPS and g + 2 >= 3:
                    kvw = load_group_kvw(g + 2)
                    for st in range(2):
                        emit_kvb_st(g + 2, st, *kvw, ckv)

            emit_norm_reduce()
            emit_norm_apply()

            # ========= Phase C: partial output projection, out^T layout =====
            for nt in range(HID // 256):
                owt = ow_pre if nt == 0 else load_ow(nt)
                for ntl in range(2):
                    pA = ps_main.tile([128, 512], F32, tag="s", name="pA")
                    pB = ps_main.tile([128, 512], F32, tag="s", name="pB")
                    for hc in range(HG):
                        lhs = owt[:, hc, ntl * 128:(ntl + 1) * 128]
                        nc.tensor.matmul(pA[:], lhs, outs_sb[:, hc, 0:512],
                                         start=(hc == 0), stop=(hc == HG - 1))
                        nc.tensor.matmul(pB[:], lhs, outs_sb[:, hc, 512:1024],
                                         start=(hc == 0), stop=(hc == HG - 1))
                    for half, pp in ((0, pA), (1, pB)):
                        osb = sbo.tile([128, 512], BF16, tag="osb", name="osb")
                        nc.scalar.copy(osb[:], pp[:])
                        nc.sync.dma_start(
                            outT.ap()[nt * 256 + ntl * 128:nt * 256 + (ntl + 1) * 128,
                                      half * 512:(half + 1) * 512], osb[:])

    nc.compile()
    return nc


def _host_inputs(hidden_states, position_ids, q_a_weight, q_a_layernorm_weight,
                 q_b_weight, kv_a_weight, kv_a_layernorm_weight, kv_b_weight,
                 o_weight):
    bf16 = ml_dtypes.bfloat16
    x = np.asarray(hidden_states, np.float32).reshape(S, HID)
    pos = np.asarray(position_ids, np.float64).reshape(S)
    q_a_w = np.asarray(q_a_weight, np.float32)
    q_ln = np.asarray(q_a_layernorm_weight, np.float32)
    q_b_w = np.asarray(q_b_weight, np.float32)
    kv_a_w = np.asarray(kv_a_weight, np.float32)
    kv_ln = np.asarray(kv_a_layernorm_weight, np.float32)
    kv_b_w = np.asarray(kv_b_weight, np.float32)
    o_w = np.asarray(o_weight, np.float32)

    wa = np.concatenate([q_a_w, kv_a_w], axis=1)               # [HID, 2112]
    xT = np.ascontiguousarray(x.T).astype(bf16)                # [HID, S]

    # per-core 320-wide wa column slabs (cores 0-6: 256 owned + 64 pad)
    slabs = np.zeros((N_CORES, HID, W_SL), np.float32)
    for c in range(7):
        slabs[c, :, 0:256] = wa[:, c * 256:(c + 1) * 256]
    slabs[7] = wa[:, 1792:2112]

    # fold the rms-norm weights into the b-projections
    qb = (q_ln[:, None] * q_b_w).reshape(CQ, H, D_Q)
    kvb = (kv_ln[:, None] * kv_b_w).reshape(CKV, H, D_NOPE + D_V)

    # rope tables
    inv_freq = 1.0 / (10000.0 ** (np.arange(0, D_ROPE, 2, dtype=np.float64) / D_ROPE))
    freqs = pos[:, None] * inv_freq[None, :]                # [S, 32]
    emb = np.concatenate([freqs, freqs], axis=-1)           # [S, 64]
    cos = np.cos(emb).astype(np.float32)
    sin = np.sin(emb).astype(np.float32)
    sin_sg = np.concatenate([-sin[:, :32], sin[:, 32:]], axis=1)  # [S, 64]
    cosT = np.ascontiguousarray(cos.T)                      # [64, S]
    sinT_sg = np.ascontiguousarray(sin_sg.T)                # [64, S]
    cos2t = np.concatenate([cosT, cosT], axis=0)            # [128, S]
    sin2tg = np.concatenate([sinT_sg, sinT_sg], axis=0)     # [128, S]
    cos_id = np.ones((D_ROPE, S), np.float32)
    sin_id = np.zeros((D_ROPE, S), np.float32)

    # causal triangle for the diagonal 128x128 blocks: key row r valid for
    # query col c iff r <= c
    i = np.arange(128)[:, None]
    j = np.arange(128)[None, :]
    tri = (i <= j).astype(np.float32).astype(bf16)

    # which cores own q columns (0-5) vs kv columns (6-7)
    coremask = np.zeros((N_CORES, 2), np.float32)
    coremask[:6, 0] = 1.0
    coremask[6:, 1] = 1.0

    in_maps = []
    for c in range(N_CORES):
        hs = slice(c * HG, (c + 1) * HG)
        in_maps.append({
            "xT": xT,
            "wsl": slabs[c].astype(bf16),
            "qbn": np.ascontiguousarray(
                qb[:, hs, :D_NOPE].reshape(CQ, HG * D_NOPE)).astype(bf16),
            "qbp": np.ascontiguousarray(
                qb[:, hs, D_NOPE:].reshape(CQ, HG * D_ROPE)).astype(bf16),
            "kvbk": np.ascontiguousarray(
                kvb[:, hs, :D_NOPE].reshape(CKV, HG * D_NOPE)).astype(bf16),
            "kvbv": np.ascontiguousarray(
                kvb[:, hs, D_NOPE:].reshape(CKV, HG * D_V)).astype(bf16),
            "ow": np.ascontiguousarray(
                o_w[c * HG * D_V:(c + 1) * HG * D_V, :]).astype(bf16),
            "cos2t": cos2t,
            "sin2tg": sin2tg,
            "cosC": cosT if c == 7 else cos_id,
            "sinC": sinT_sg if c == 7 else sin_id,
            "tri": tri,
            "coremask": coremask,
        })
    return in_maps


def kernel(**inputs):
    global LAST_EXEC_NS
    trace = bool(inputs.pop("_trace", False))
    in_maps = _host_inputs(**inputs)
    if "nc" not in _CACHE:
        _CACHE["nc"] = _build_nc()
    nc = _CACHE["nc"]
    res = bass_utils.run_bass_kernel_spmd(
        nc, in_maps, core_ids=list(range(N_CORES)), trace=trace)
    LAST_EXEC_NS = res.exec_time_ns
    total = np.zeros((HID, S), np.float64)
    for c in range(N_CORES):
        total += res.results[c]["outT"].astype(np.float64)
    return np.ascontiguousarray(total.T).astype(np.float32).reshape(1, 1, S, HID)


# revision 31
# speedup vs baseline: 1.0953x; 1.0902x over previous
"""DeepseekV3 MLA attention prefill (S=1024, H=128 heads, HID=7168) on 8 TRN2
NeuronCores.

Sharding: tensor-parallel over heads (16 heads/core) for attention and the
output projection (host sums 8 partial outT). The fused a-projection is
COLUMN-sharded (v3): each core computes all 1024 rows for its own 320-column
slab of [q_a | kv_a | k_pe] (cores 0-5 own q columns, 6-7 own kv + k_pe,
64 pad columns on 0-6), so only ~19MB (full x^T + its wa slab) moves per
core instead of the 33MB replicated-wa scheme. The rms denominators cross
cores via an 8KB AllReduce; activations are produced directly in the
transposed [col, s] layout (stationary = wa chunks), so no PE transposes are
needed before the s-split AllGather pair.

Softmax normalization never touches the PE: gpsimd.partition_all_reduce +
fast DVE reciprocal + multiply, deferred one group. Causal windows start at
the diagonal. AV matmuls run one kc-step behind the score matmuls. kv_b
projections for 3 groups run right after the AllGather to cover its tail;
later groups are produced 2 groups ahead inside the loop.
"""
import math
import numpy as np
import ml_dtypes

import concourse.bass as bass
import concourse.mybir as mybir
import concourse.bacc as bacc
import concourse.bass_isa as bass_isa
import concourse.tile as tile
import concourse.bass_utils as bass_utils
from contextlib import ExitStack

F32 = mybir.dt.float32
BF16 = mybir.dt.bfloat16
AF = mybir.ActivationFunctionType
OP = mybir.AluOpType
RED = bass_isa.ReduceOp

N_CORES = 8
S = 1024
HID = 7168
H = 128
HG = H // N_CORES          # 16 heads per core
D_NOPE = 128
D_ROPE = 64
D_Q = D_NOPE + D_ROPE      # 192
D_V = 128
CQ = 1536                  # q lora rank
CKV = 512                  # kv lora rank
CA = CQ + CKV + D_ROPE     # 2112 fused a-proj cols
W_SL = 320                 # per-core a-proj column slab (256 owned + pad)
CC_A = HID // 128          # 56 contraction chunks for a-proj
SCALE = 1.0 / math.sqrt(D_Q)
EPS = 1e-6
G_HEADS = 2                # heads per group
N_GROUPS = HG // G_HEADS   # 8 groups
LAST_EXEC_NS = None

_CACHE = {}


def _dma_rows_to_3d(nc, dst, src_ap, n_chunks, p=128):
    """dst [p, n_chunks, w] <- src rows laid out as (chunk, p)."""
    try:
        nc.sync.dma_start(dst, src_ap.rearrange("(c p) s -> p c s", p=p))
    except Exception:
        for c in range(n_chunks):
            nc.sync.dma_start(dst[:, c, :], src_ap[c * p:(c + 1) * p, :])


def _build_nc():
    nc = bacc.Bacc("TRN2", target_bir_lowering=False, debug=False,
                   num_devices=N_CORES)

    xT = nc.dram_tensor("xT", [HID, S], BF16, kind="ExternalInput")
    wsl = nc.dram_tensor("wsl", [HID, W_SL], BF16, kind="ExternalInput")
    qbn = nc.dram_tensor("qbn", [CQ, HG * D_NOPE], BF16, kind="ExternalInput")
    qbp = nc.dram_tensor("qbp", [CQ, HG * D_ROPE], BF16, kind="ExternalInput")
    kvbk = nc.dram_tensor("kvbk", [CKV, HG * D_NOPE], BF16, kind="ExternalInput")
    kvbv = nc.dram_tensor("kvbv", [CKV, HG * D_V], BF16, kind="ExternalInput")
    ow = nc.dram_tensor("ow", [HG * D_V, HID], BF16, kind="ExternalInput")
    cos2t = nc.dram_tensor("cos2t", [128, S], F32, kind="ExternalInput")
    sin2tg = nc.dram_tensor("sin2tg", [128, S], F32, kind="ExternalInput")
    cosC = nc.dram_tensor("cosC", [D_ROPE, S], F32, kind="ExternalInput")
    sinC = nc.dram_tensor("sinC", [D_ROPE, S], F32, kind="ExternalInput")
    tri = nc.dram_tensor("tri", [128, 128], BF16, kind="ExternalInput")
    coremask = nc.dram_tensor("coremask", [N_CORES, 2], F32,
                               kind="ExternalInput")
    outT = nc.dram_tensor("outT", [HID, S], BF16, kind="ExternalOutput")

    with tile.TileContext(nc) as tc, ExitStack() as top:
        const = top.enter_context(tc.tile_pool(name="const", bufs=1))
        dram = top.enter_context(tc.tile_pool(name="dram", bufs=1, space="DRAM"))
        outsp = top.enter_context(tc.tile_pool(name="outsp", bufs=1))
        sbwq = top.enter_context(tc.tile_pool(name="sbwq", bufs=2))
        sbow = top.enter_context(tc.tile_pool(name="sbow", bufs=2))
        sbkw = top.enter_context(tc.tile_pool(name="sbkw", bufs=3))
        sbkv = top.enter_context(tc.tile_pool(name="sbkv", bufs=3))
        sbg = top.enter_context(tc.tile_pool(name="sbg", bufs=1))
        sbf = top.enter_context(tc.tile_pool(name="sbf", bufs=1))

        # ---- constants in SBUF ----
        tri_sb = const.tile([128, 128], BF16, tag="tri")
        nc.sync.dma_start(tri_sb[:], tri.ap())
        cos2t_sb = const.tile([128, S], F32, tag="cos2t")
        sin2tg_sb = const.tile([128, S], F32, tag="sin2tg")
        nc.sync.dma_start(cos2t_sb[:], cos2t.ap())
        nc.sync.dma_start(sin2tg_sb[:], sin2tg.ap())
        cosC_sb = const.tile([D_ROPE, S], F32, tag="cosC")
        sinC_sb = const.tile([D_ROPE, S], F32, tag="sinC")
        nc.sync.dma_start(cosC_sb[:], cosC.ap())
        nc.sync.dma_start(sinC_sb[:], sinC.ap())
        coremask_sb = const.tile([N_CORES, 2], F32, tag="coremask")
        nc.sync.dma_start(coremask_sb[:], coremask.ap())

        agS = dram.tile([1, S], F32, tag="agS")
        agoS = dram.tile([N_CORES, S], F32, tag="agoS", addr_space="Shared")
        agiA = dram.tile([W_SL, 512], BF16, tag="agiA")
        agiB = dram.tile([W_SL, 512], BF16, tag="agiB")
        agoA = dram.tile([W_SL * N_CORES, 512], BF16, tag="agoA",
                         addr_space="Shared")
        agoB = dram.tile([W_SL * N_CORES, 512], BF16, tag="agoB",
                         addr_space="Shared")

        # all 16 heads' attention outputs live in SBUF [dv=128, head, s]
        outs_sb = outsp.tile([128, HG, S], BF16, tag="outs")

        def load_group_qw(g):
            h0 = g * G_HEADS
            qbnw = sbwq.tile([128, CQ // 128, G_HEADS * 128], BF16,
                             tag="qbnw", name="qbnw")
            qbpw = sbwq.tile([128, CQ // 128, G_HEADS * 64], BF16,
                             tag="qbpw", name="qbpw")
            _dma_rows_to_3d(nc, qbnw[:],
                            qbn.ap()[:, h0 * 128:(h0 + G_HEADS) * 128], CQ // 128)
            _dma_rows_to_3d(nc, qbpw[:],
                            qbp.ap()[:, h0 * 64:(h0 + G_HEADS) * 64], CQ // 128)
            return qbnw, qbpw

        def load_group_kvw(g):
            h0 = g * G_HEADS
            kvbkw = sbkw.tile([128, CKV // 128, G_HEADS * 128], BF16,
                              tag="kvbkw", name="kvbkw")
            kvbvw = sbkw.tile([128, CKV // 128, G_HEADS * 128], BF16,
                              tag="kvbvw", name="kvbvw")
            _dma_rows_to_3d(nc, kvbkw[:],
                            kvbk.ap()[:, h0 * 128:(h0 + G_HEADS) * 128], CKV // 128)
            _dma_rows_to_3d(nc, kvbvw[:],
                            kvbv.ap()[:, h0 * 128:(h0 + G_HEADS) * 128], CKV // 128)
            return kvbkw, kvbvw

        def load_ow(nt):
            owt = sbow.tile([128, HG, 256], BF16, tag="owt", name="owt")
            _dma_rows_to_3d(nc, owt[:],
                            ow.ap()[:, nt * 256:(nt + 1) * 256], HG)
            return owt

        # kv_b output tiles: 3-slot sliding window (group g+2 is produced at
        # the end of group g's attention)
        kv_tiles = {}

        def emit_kvb_st(g, st, kvbkw, kvbvw, ckv):
            if g not in kv_tiles:
                kT_g = sbkv.tile([128, G_HEADS, S], BF16, tag="kT",
                                 name=f"kT{g}")
                v_g = sbkv.tile([128, 8, G_HEADS * 128], BF16, tag="v",
                                name=f"v{g}")
                kv_tiles[g] = (kT_g, v_g)
            kT_g, v_g = kv_tiles[g]
            for i in range(G_HEADS):
                p = ps_main.tile([128, 512], F32, tag="s", name="pkv")
                for c in range(CKV // 128):
                    nc.tensor.matmul(p[:], kvbkw[:, c, i * 128:(i + 1) * 128],
                                     ckv[st][:, c, :],
                                     start=(c == 0), stop=(c == CKV // 128 - 1))
                nc.scalar.copy(kT_g[:, i, st * 512:(st + 1) * 512], p[:])
            for sc in range(st * 4, st * 4 + 4):
                p = ps_main.tile([128, 512], F32, tag="s", name="pkv")
                nn = G_HEADS * 128
                for c in range(CKV // 128):
                    nc.tensor.matmul(
                        p[:, :nn],
                        ckv[st][:, c, (sc % 4) * 128:(sc % 4 + 1) * 128],
                        kvbvw[:, c, :],
                        start=(c == 0), stop=(c == CKV // 128 - 1))
                nc.scalar.copy(v_g[:, sc, :], p[:, :nn])

        # ================= Phase A: column-sharded a-proj ==================
        with ExitStack() as pa:
            sba = pa.enter_context(tc.tile_pool(name="sba", bufs=1))
            sbst = pa.enter_context(tc.tile_pool(name="sbst", bufs=4))
            psa = pa.enter_context(tc.tile_pool(name="psa", bufs=1, space="PSUM"))

            # psum accumulators: [col, s] layout, held across the whole pass
            pAc = [psa.tile([128, 512], F32, tag=f"pa{j}", name=f"pa{j}")
                   for j in range(4)]            # A-s0, A-s1, B-s0, B-s1
            pCc = [psa.tile([64, 512], F32, tag=f"pc{j}", name=f"pc{j}")
                   for j in range(2)]            # C-s0, C-s1

            for cc in range(CC_A):
                xt_t = sbst.tile([128, S], BF16, tag="xt", name="xt_t")
                nc.sync.dma_start(xt_t[:], xT.ap()[cc * 128:(cc + 1) * 128, :])
                ws_t = sbst.tile([128, W_SL], BF16, tag="ws", name="ws_t")
                nc.sync.dma_start(ws_t[:], wsl.ap()[cc * 128:(cc + 1) * 128, :])
                st_, sp_ = (cc == 0), (cc == CC_A - 1)
                for half in range(2):
                    sl = slice(half * 512, (half + 1) * 512)
                    nc.tensor.matmul(pAc[half][:], ws_t[:, 0:128], xt_t[:, sl],
                                     start=st_, stop=sp_)
                    nc.tensor.matmul(pAc[2 + half][:], ws_t[:, 128:256],
                                     xt_t[:, sl], start=st_, stop=sp_)
                    nc.tensor.matmul(pCc[half][:], ws_t[:, 256:320],
                                     xt_t[:, sl], start=st_, stop=sp_)

            # ---- rms denominators: each core's A/B columns are homogeneous
            # (all-q or all-kv), so a plain ones-column reduce gives this
            # core's own sums; routing to q vs kv happens receiver-side ----
            sqA = sba.tile([128, S], BF16, tag="sqA")
            sqB = sba.tile([128, S], BF16, tag="sqB")
            for half in range(2):
                sl = slice(half * 512, (half + 1) * 512)
                nc.scalar.activation(sqA[:, sl], pAc[half][:], AF.Square)
                nc.scalar.activation(sqB[:, sl], pAc[2 + half][:], AF.Square)
            psS = [psa.tile([1, 512], F32, tag=f"ss{j}", name=f"ss{j}")
                   for j in range(2)]
            for half in range(2):
                sl = slice(half * 512, (half + 1) * 512)
                nc.tensor.matmul(psS[half][:], tri_sb[:, 127:128], sqA[:, sl],
                                 start=True, stop=False)
                nc.tensor.matmul(psS[half][:], tri_sb[:, 127:128], sqB[:, sl],
                                 start=False, stop=True)
            sums_sb = sba.tile([1, S], F32, tag="sums_sb")
            for half in range(2):
                nc.scalar.copy(sums_sb[0:1, half * 512:(half + 1) * 512],
                               psS[half][:])
            nc.sync.dma_start(agS[:, :], sums_sb[:])
            nc.gpsimd.collective_compute(
                "AllGather", OP.bypass,
                replica_groups=[list(range(N_CORES))],
                ins=[agS.opt()], outs=[agoS.opt()],
            )

            # ---- bf16 casts of the (unnormalized) activations ----
            bTA = sba.tile([128, S], BF16, tag="bTA")
            bTB = sba.tile([128, S], BF16, tag="bTB")
            for half in range(2):
                sl = slice(half * 512, (half + 1) * 512)
                nc.vector.tensor_copy(bTA[:, sl], pAc[half][:])
                nc.vector.tensor_copy(bTB[:, sl], pAc[2 + half][:])

            # ---- k_pe rope in transposed [d, s] layout (chunk C) ----
            # identity tables on cores 0-6 make this a no-op copy there
            kraw = sba.tile([64, S], F32, tag="kraw")
            rrC = sba.tile([64, S], F32, tag="rrC")
            bTC = sba.tile([64, S], BF16, tag="bTC")
            for half in range(2):
                sl = slice(half * 512, (half + 1) * 512)
                nc.vector.tensor_copy(kraw[:, sl], pCc[half][:])
            nc.vector.tensor_copy(rrC[0:32, :], kraw[32:64, :])
            nc.vector.tensor_copy(rrC[32:64, :], kraw[0:32, :])
            nc.vector.tensor_mul(rrC[:], rrC[:], sinC_sb[:])
            nc.vector.tensor_mul(kraw[:], kraw[:], cosC_sb[:])
            nc.vector.tensor_add(bTC[:], kraw[:], rrC[:])

            # ---- AllGather inputs (already transposed, raw) ----
            for agi, half in ((agiA, 0), (agiB, 1)):
                sl = slice(half * 512, (half + 1) * 512)
                nc.sync.dma_start(agi[0:128, :], bTA[:, sl])
                nc.sync.dma_start(agi[128:256, :], bTB[:, sl])
                nc.sync.dma_start(agi[256:320, :], bTC[:, sl])
            nc.gpsimd.collective_compute(
                "AllGather", OP.bypass,
                replica_groups=[list(range(N_CORES))],
                ins=[agiA.opt()], outs=[agoA.opt()],
            )
            nc.gpsimd.collective_compute(
                "AllGather", OP.bypass,
                replica_groups=[list(range(N_CORES))],
                ins=[agiB.opt()], outs=[agoB.opt()],
            )

            # prefetches (FIFO DMA queues reach these behind the agi inputs,
            # well before the AllGathers complete)
            kvw_q = [load_group_kvw(0), load_group_kvw(1)]
            wq_pre = [load_group_qw(0), load_group_qw(1)]
            ow_pre = load_ow(0)

        ps_main = top.enter_context(tc.tile_pool(name="ps_main", bufs=3,
                                                 space="PSUM"))

        # ---- rms factors from the gathered per-core sums (overlaps the big
        # AllGathers): mask by core type, partition-reduce, rsqrt, broadcast.
        # fq_bc/fk_bc persist: the q factor is folded into every qTn/qp psum
        # copy; the kv factor is applied once to the stitched ckv tiles.
        fq_bc = sbf.tile([128, S], F32, tag="fq_bc")
        fk_bc = sbf.tile([128, S], F32, tag="fk_bc")
        with ExitStack() as pf:
            sbr = pf.enter_context(tc.tile_pool(name="sbr", bufs=1))
            sums8 = sbr.tile([N_CORES, S], F32, tag="sums8")
            nc.sync.dma_start(sums8[:], agoS[:, :])
            tq8 = sbr.tile([N_CORES, S], F32, tag="tq8")
            tk8 = sbr.tile([N_CORES, S], F32, tag="tk8")
            nc.vector.tensor_scalar_mul(tq8[:], sums8[:], coremask_sb[:, 0:1])
            nc.vector.tensor_scalar_mul(tk8[:], sums8[:], coremask_sb[:, 1:2])
            dq8 = sbr.tile([N_CORES, S], F32, tag="dq8")
            dk8 = sbr.tile([N_CORES, S], F32, tag="dk8")
            nc.gpsimd.partition_all_reduce(dq8[:], tq8[:], N_CORES, RED.add)
            nc.gpsimd.partition_all_reduce(dk8[:], tk8[:], N_CORES, RED.add)
            for d8, n in ((dq8, CQ), (dk8, CKV)):
                nc.vector.tensor_scalar(d8[0:1, :], d8[0:1, :], 1.0 / n, EPS,
                                        OP.mult, OP.add)
                nc.vector.reciprocal_approx_fast(d8[0:1, :], d8[0:1, :])
                nc.scalar.activation(d8[0:1, :], d8[0:1, :], AF.Sqrt)
            nc.gpsimd.partition_broadcast(fq_bc[:], dq8[0:1, :])
            nc.gpsimd.partition_broadcast(fk_bc[:], dk8[0:1, :])

        # ---- stitch the gathered activations ----
        # global row of col j on core c is c*320 + j; q cols sit on cores
        # 0-5 (2 aligned 128-chunks each), kv on 6-7, k_pe on core 7 rows
        # 256:320 (already rope'd, transposed)
        qct = []
        ckv = []
        kpe2 = sbg.tile([128, S], BF16, tag="kpe2")
        for st, ago in ((0, agoA), (1, agoB)):
            k_t = sbg.tile([128, CKV // 128, 512], BF16, tag=f"ckv{st}",
                           name=f"ckv{st}")
            for c in range(CKV // 128):
                base = (6 + c // 2) * W_SL + (c % 2) * 128
                nc.sync.dma_start(k_t[:, c, :], ago[base:base + 128, :])
            # fold the kv rms factor into the stitched tiles once
            for c in range(CKV // 128):
                nc.vector.tensor_mul(k_t[:, c, :], k_t[:, c, :],
                                     fk_bc[:, st * 512:(st + 1) * 512])
            ckv.append(k_t)
            base = 7 * W_SL + 256
            nc.sync.dma_start(kpe2[0:64, st * 512:(st + 1) * 512],
                              ago[base:base + 64, :])
            nc.sync.dma_start(kpe2[64:128, st * 512:(st + 1) * 512],
                              ago[base:base + 64, :])
        for st, ago in ((0, agoA), (1, agoB)):
            q_t = sbg.tile([128, CQ // 128, 512], BF16, tag=f"qct{st}",
                           name=f"qct{st}")
            for c in range(CQ // 128):
                base = (c // 2) * W_SL + (c % 2) * 128
                nc.sync.dma_start(q_t[:, c, :], ago[base:base + 128, :])
            qct.append(q_t)

        # kv_b for groups 0-2 covers the AllGather tail (st0 chains first so
        # they only wait on the first AllGather)
        kvw_q.append(load_group_kvw(2))
        for st in range(2):
            for g in range(3):
                emit_kvb_st(g, st, *kvw_q[g], ckv)

        # ================= Phase B: q_b projections + attention =============
        with ExitStack() as pb:
            sbh = pb.enter_context(tc.tile_pool(name="sbh", bufs=2))
            sbp = pb.enter_context(tc.tile_pool(name="sbp", bufs=1))
            sbpt = pb.enter_context(tc.tile_pool(name="sbpt", bufs=5))
            sbs = pb.enter_context(tc.tile_pool(name="sbs", bufs=2))
            sbn = pb.enter_context(tc.tile_pool(name="sbn", bufs=2))
            sbo = pb.enter_context(tc.tile_pool(name="sbo", bufs=3))
            ps_o = pb.enter_context(tc.tile_pool(name="ps_o", bufs=2, space="PSUM"))

            pending_norm = []

            def emit_norm_reduce():
                for idx, (h_idx, qt_, sums_, psum_o_) in enumerate(pending_norm):
                    den = sbn.tile([128, 512], F32, tag="den", name="den")
                    nc.gpsimd.partition_all_reduce(den[:], sums_[:], 128, RED.add)
                    pending_norm[idx] = (h_idx, qt_, den, psum_o_)

            def emit_norm_apply():
                while pending_norm:
                    h_idx, qt_, den, psum_o_ = pending_norm.pop(0)
                    rec = sbn.tile([128, 512], F32, tag="rec", name="rec")
                    nc.vector.reciprocal_approx_fast(rec[:], den[:])
                    nc.vector.tensor_mul(
                        outs_sb[:, h_idx, qt_ * 512:(qt_ + 1) * 512],
                        psum_o_[:], rec[:])

            for g in range(N_GROUPS):
                h0 = g * G_HEADS
                qbnw, qbpw = wq_pre[g] if g < 2 else load_group_qw(g)

                # --- q rope projection first so the DVE rope work is done
                # before the first rope-score matmul needs qTp ---
                qp_raw = sbp.tile([128, S], F32, tag="qp_raw", name="qp_raw")
                p0 = ps_main.tile([128, 512], F32, tag="s", name="p0")
                p1 = ps_main.tile([128, 512], F32, tag="s", name="p1")
                for c in range(CQ // 128):
                    nc.tensor.matmul(p0[:], qbpw[:, c, :], qct[0][:, c, :],
                                     start=(c == 0), stop=(c == CQ // 128 - 1))
                    nc.tensor.matmul(p1[:], qbpw[:, c, :], qct[1][:, c, :],
                                     start=(c == 0), stop=(c == CQ // 128 - 1))
                nc.vector.tensor_mul(qp_raw[:, 0:512], p0[:], fq_bc[:, 0:512])
                nc.vector.tensor_mul(qp_raw[:, 512:1024], p1[:],
                                     fq_bc[:, 512:1024])
                emit_norm_reduce()   # prev group's partition reduces (GpSimd)
                # rope on the head-pair tile: rows [0:64]=head h0, [64:128]=h0+1
                qTp = sbh.tile([128, S], BF16, tag="qTp")
                rs = sbp.tile([128, S], F32, tag="ropes")
                for hh in range(2):
                    sl = slice(hh * 512, (hh + 1) * 512)
                    for b in range(4):
                        r0 = b * 32
                        r1 = r0 + 32 if b % 2 == 0 else r0 - 32
                        nc.vector.tensor_copy(rs[r0:r0 + 32, sl], qp_raw[r1:r1 + 32, sl])
                    nc.vector.tensor_mul(rs[:, sl], rs[:, sl], sin2tg_sb[:, sl])
                    nc.vector.tensor_mul(qp_raw[:, sl], qp_raw[:, sl], cos2t_sb[:, sl])
                    nc.vector.tensor_add(qTp[:, sl], qp_raw[:, sl], rs[:, sl])

                # --- q nope projections; st-paired so each stationary is
                # loaded once per two 512-streams ---
                qTn = []
                for i in range(G_HEADS):
                    qt_t = sbh.tile([128, S], BF16, tag=f"qTn{i}", name=f"qTn{i}")
                    p0 = ps_main.tile([128, 512], F32, tag="s", name="p0")
                    p1 = ps_main.tile([128, 512], F32, tag="s", name="p1")
                    if g == 0 and i == 0:
                        # first chains after the AllGather: unpaired, so the
                        # st=0 chain starts as soon as half the stitch lands
                        for c in range(CQ // 128):
                            nc.tensor.matmul(p0[:], qbnw[:, c, 0:128],
                                             qct[0][:, c, :],
                                             start=(c == 0), stop=(c == CQ // 128 - 1))
                        for c in range(CQ // 128):
                            nc.tensor.matmul(p1[:], qbnw[:, c, 0:128],
                                             qct[1][:, c, :],
                                             start=(c == 0), stop=(c == CQ // 128 - 1))
                    else:
                        for c in range(CQ // 128):
                            nc.tensor.matmul(p0[:], qbnw[:, c, i * 128:(i + 1) * 128],
                                             qct[0][:, c, :],
                                             start=(c == 0), stop=(c == CQ // 128 - 1))
                            nc.tensor.matmul(p1[:], qbnw[:, c, i * 128:(i + 1) * 128],
                                             qct[1][:, c, :],
                                             start=(c == 0), stop=(c == CQ // 128 - 1))
                    nc.vector.tensor_mul(qt_t[:, 0:512], p0[:], fq_bc[:, 0:512])
                    nc.vector.tensor_mul(qt_t[:, 512:1024], p1[:],
                                         fq_bc[:, 512:1024])
                    qTn.append(qt_t)
                    if i == 0:
                        emit_norm_apply()  # prev group's recip+mul (DVE)

                kT_g, v_g = kv_tiles[g]

                # --- attention: heads interleaved, AV skewed one kc behind,
                # causal windows start at the diagonal ---
                for qt in range(2):
                    kmax = 4 * (qt + 1)
                    sums = [sbs.tile([128, 512], F32, tag=f"sums{i}", name=f"sums{i}")
                            for i in range(G_HEADS)]
                    psum_o = [ps_o.tile([128, 512], F32, tag=f"o{i}", name=f"po{i}")
                              for i in range(G_HEADS)]
                    pt = {}

                    def av_step(kc, last):
                        offp = max(0, (kc - 4 * qt)) * 128
                        for i in range(G_HEADS):
                            nc.tensor.matmul(psum_o[i][:, offp:512],
                                             v_g[:, kc, i * 128:(i + 1) * 128],
                                             pt[(i, kc)][:, offp:512],
                                             start=(kc == 0), stop=last,
                                             skip_group_check=True)

                    for kc in range(kmax):
                        off = max(0, (kc - 4 * qt)) * 128
                        qsl = slice(qt * 512 + off, (qt + 1) * 512)
                        for i in range(G_HEADS):
                            ps = ps_main.tile([128, 512], F32, tag="s", name="ps")
                            nc.tensor.matmul(ps[:, off:512],
                                             kT_g[:, i, kc * 128:(kc + 1) * 128],
                                             qTn[i][:, qsl],
                                             start=True, stop=False)
                            nc.tensor.matmul(ps[:, off:512],
                                             kpe2[i * 64:(i + 1) * 64,
                                                  kc * 128:(kc + 1) * 128],
                                             qTp[i * 64:(i + 1) * 64, qsl],
                                             start=False, stop=True)
                            p = sbpt.tile([128, 512], BF16, tag="pt", name="p")
                            nc.scalar.activation(p[:, off:512], ps[:, off:512],
                                                 AF.Exp, scale=SCALE)
                            if kc >= 4 * qt:
                                nc.vector.tensor_mul(p[:, off:off + 128],
                                                     p[:, off:off + 128], tri_sb[:])
                            if kc == 0:
                                nc.vector.tensor_copy(sums[i][:], p[:])
                            else:
                                nc.vector.tensor_add(sums[i][:, off:512],
                                                     sums[i][:, off:512],
                                                     p[:, off:512])
                            pt[(i, kc)] = p
                        if kc > 0:
                            av_step(kc - 1, last=False)
                    av_step(kmax - 1, last=True)
                    for i in range(G_HEADS):
                        pending_norm.append((h0 + i, qt, sums[i], psum_o[i]))

                # produce kv for group g+2 (slides the 3-buffer window)
                if g + 2 < N_GROUPS and g + 2 >= 3:
                    kvw = load_group_kvw(g + 2)
                    for st in range(2):
                        emit_kvb_st(g + 2, st, *kvw, ckv)

            emit_norm_reduce()
            emit_norm_apply()

            # ========= Phase C: partial output projection, out^T layout =====
            for nt in range(HID // 256):
                owt = ow_pre if nt == 0 else load_ow(nt)
                for ntl in range(2):
                    pA = ps_main.tile([128, 512], F32, tag="s", name="pA")
                    pB = ps_main.tile([128, 512], F32, tag="s", name="pB")
                    for hc in range(HG):
                        lhs = owt[:, hc, ntl * 128:(ntl + 1) * 128]
                        nc.tensor.matmul(pA[:], lhs, outs_sb[:, hc, 0:512],
                                         start=(hc == 0), stop=(hc == HG - 1))
                        nc.tensor.matmul(pB[:], lhs, outs_sb[:, hc, 512:1024],
                                         start=(hc == 0), stop=(hc == HG - 1))
                    for half, pp in ((0, pA), (1, pB)):
                        osb = sbo.tile([128, 512], BF16, tag="osb", name="osb")
                        nc.scalar.copy(osb[:], pp[:])
                        nc.sync.dma_start(
                            outT.ap()[nt * 256 + ntl * 128:nt * 256 + (ntl + 1) * 128,
                                      half * 512:(half + 1) * 512], osb[:])

    nc.compile()
    return nc


def _host_inputs(hidden_states, position_ids, q_a_weight, q_a_layernorm_weight,
                 q_b_weight, kv_a_weight, kv_a_layernorm_weight, kv_b_weight,
                 o_weight):
    bf16 = ml_dtypes.bfloat16
    x = np.asarray(hidden_states, np.float32).reshape(S, HID)
    pos = np.asarray(position_ids, np.float64).reshape(S)
    q_a_w = np.asarray(q_a_weight, np.float32)
    q_ln = np.asarray(q_a_layernorm_weight, np.float32)
    q_b_w = np.asarray(q_b_weight, np.float32)
    kv_a_w = np.asarray(kv_a_weight, np.float32)
    kv_ln = np.asarray(kv_a_layernorm_weight, np.float32)
    kv_b_w = np.asarray(kv_b_weight, np.float32)
    o_w = np.asarray(o_weight, np.float32)

    wa = np.concatenate([q_a_w, kv_a_w], axis=1)               # [HID, 2112]
    xT = np.ascontiguousarray(x.T).astype(bf16)                # [HID, S]

    # per-core 320-wide wa column slabs (cores 0-6: 256 owned + 64 pad)
    slabs = np.zeros((N_CORES, HID, W_SL), np.float32)
    for c in range(7):
        slabs[c, :, 0:256] = wa[:, c * 256:(c + 1) * 256]
    slabs[7] = wa[:, 1792:2112]

    # fold the rms-norm weights into the b-projections
    qb = (q_ln[:, None] * q_b_w).reshape(CQ, H, D_Q)
    kvb = (kv_ln[:, None] * kv_b_w).reshape(CKV, H, D_NOPE + D_V)

    # rope tables
    inv_freq = 1.0 / (10000.0 ** (np.arange(0, D_ROPE, 2, dtype=np.float64) / D_ROPE))
    freqs = pos[:, None] * inv_freq[None, :]                # [S, 32]
    emb = np.concatenate([freqs, freqs], axis=-1)           # [S, 64]
    cos = np.cos(emb).astype(np.float32)
    sin = np.sin(emb).astype(np.float32)
    sin_sg = np.concatenate([-sin[:, :32], sin[:, 32:]], axis=1)  # [S, 64]
    cosT = np.ascontiguousarray(cos.T)                      # [64, S]
    sinT_sg = np.ascontiguousarray(sin_sg.T)                # [64, S]
    cos2t = np.concatenate([cosT, cosT], axis=0)            # [128, S]
    sin2tg = np.concatenate([sinT_sg, sinT_sg], axis=0)     # [128, S]
    cos_id = np.ones((D_ROPE, S), np.float32)
    sin_id = np.zeros((D_ROPE, S), np.float32)

    # causal triangle for the diagonal 128x128 blocks: key row r valid for
    # query col c iff r <= c
    i = np.arange(128)[:, None]
    j = np.arange(128)[None, :]
    tri = (i <= j).astype(np.float32).astype(bf16)

    # which cores own q columns (0-5) vs kv columns (6-7)
    coremask = np.zeros((N_CORES, 2), np.float32)
    coremask[:6, 0] = 1.0
    coremask[6:, 1] = 1.0

    in_maps = []
    for c in range(N_CORES):
        hs = slice(c * HG, (c + 1) * HG)
        in_maps.append({
            "xT": xT,
            "wsl": slabs[c].astype(bf16),
            "qbn": np.ascontiguousarray(
                qb[:, hs, :D_NOPE].reshape(CQ, HG * D_NOPE)).astype(bf16),
            "qbp": np.ascontiguousarray(
                qb[:, hs, D_NOPE:].reshape(CQ, HG * D_ROPE)).astype(bf16),
            "kvbk": np.ascontiguousarray(
                kvb[:, hs, :D_NOPE].reshape(CKV, HG * D_NOPE)).astype(bf16),
            "kvbv": np.ascontiguousarray(
                kvb[:, hs, D_NOPE:].reshape(CKV, HG * D_V)).astype(bf16),
            "ow": np.ascontiguousarray(
                o_w[c * HG * D_V:(c + 1) * HG * D_V, :]).astype(bf16),
            "cos2t": cos2t,
            "sin2tg": sin2tg,
            "cosC": cosT if c == 7 else cos_id,
            "sinC": sinT_sg if c == 7 else sin_id,
            "tri": tri,
            "coremask": coremask,
        })
    return in_maps


def kernel(**inputs):
    global LAST_EXEC_NS
    trace = bool(inputs.pop("_trace", False))
    in_maps = _host_inputs(**inputs)
    if "nc" not in _CACHE:
        _CACHE["nc"] = _build_nc()
    nc = _CACHE["nc"]
    res = bass_utils.run_bass_kernel_spmd(
        nc, in_maps, core_ids=list(range(N_CORES)), trace=trace)
    LAST_EXEC_NS = res.exec_time_ns
    total = np.zeros((HID, S), np.float64)
    for c in range(N_CORES):
        total += res.results[c]["outT"].astype(np.float64)
    return np.ascontiguousarray(total.T).astype(np.float32).reshape(1, 1, S, HID)
